# revision 17
# baseline (speedup 1.0000x reference)
"""Trainium2 Bass kernel for DifferentiableRobotModel forward kinematics.

Math: the chain quat  chain_i = qf_0 (x) Z(q_0) (x) qf_1 (x) Y(q_1) ...  is
evaluated as a sweep of single-axis rotations.  Each fixed rotation qf_i is
Euler-decomposed (host, float64) as A1(a_i) X(b_i) A3(g_i) where A3 = joint
axis of link i (z even / y odd) and A1 = joint axis of link i-1, so the
whole chain is:  per link: [A1(a_i)] [X(b_i)] [A3(g_i + q_i)] — two
constant-angle rotations + one variable-angle rotation.  A single-axis
right-multiplication on the quat state D is  D' = c*D + s*(D (x) axis_hat),
where (D (x) axis_hat) is a signed permutation of D's components —
expressible as strided access patterns, so each rotation is 3-4 fused DVE
ops on [128, 4*512] tiles.

Translations:  t_i = t_{i-1} + R(chain_{i-1}) @ tf_i  via the quat-rotate
identity  R(D)v = 2w(u x v) + 2u(u.v) + (2w^2-1)v  (unit quat), with the
constant-scale multiplies and affine terms offloaded to the Scalar engine.

Outputs match the reference's (x,y,z,w) quats canonicalized to w >= 0.

Data parallel over 8 NeuronCores: core c owns batch slice [c*65536,(c+1)*65536),
65536 = 128 partitions x 512 free elements per core.
"""
import sys
import dataclasses

sys.path.insert(0, '/opt/trn_rl_repo')

import numpy as np

N_LINKS = 7
N_CORES = 8
BATCH = 524288
PER_CORE = BATCH // N_CORES          # 65536
P = 128
F = PER_CORE // P                    # 512
PI = float(np.pi)
# joint axes: z for even links, y for odd links
AXIS_IS_Z = [True, False, True, False, True, False, True]

_CACHE = {}


# ---------------------------------------------------------------- host math
def _rx(a):
    c, s = np.cos(a), np.sin(a)
    return np.array([[1, 0, 0], [0, c, -s], [0, s, c]], dtype=np.float64)


def _ry(a):
    c, s = np.cos(a), np.sin(a)
    return np.array([[c, 0, s], [0, 1, 0], [-s, 0, c]], dtype=np.float64)


def _rz(a):
    c, s = np.cos(a), np.sin(a)
    return np.array([[c, -s, 0], [s, c, 0], [0, 0, 1]], dtype=np.float64)


def _euler_yxz(R):
    # R = Ry(a) Rx(b) Rz(g)
    b = np.arcsin(np.clip(-R[1, 2], -1, 1))
    a = np.arctan2(R[0, 2], R[2, 2])
    g = np.arctan2(R[1, 0], R[1, 1])
    return a, b, g


def _euler_zxy(R):
    # R = Rz(a) Rx(b) Ry(g)
    b = np.arcsin(np.clip(R[2, 1], -1, 1))
    a = np.arctan2(-R[0, 1], R[1, 1])
    g = np.arctan2(-R[2, 0], R[2, 2])
    return a, b, g


def _quat_from_R(R):
    # float64 rotation -> quat (x, y, z, w), any sign
    t = np.trace(R)
    if t > 0:
        w = 0.5 * np.sqrt(1 + t)
        x = (R[2, 1] - R[1, 2]) / (4 * w)
        y = (R[0, 2] - R[2, 0]) / (4 * w)
        z = (R[1, 0] - R[0, 1]) / (4 * w)
    else:
        i = int(np.argmax(np.diag(R)))
        if i == 0:
            x = 0.5 * np.sqrt(1 + R[0, 0] - R[1, 1] - R[2, 2])
            w = (R[2, 1] - R[1, 2]) / (4 * x)
            y = (R[0, 1] + R[1, 0]) / (4 * x)
            z = (R[0, 2] + R[2, 0]) / (4 * x)
        elif i == 1:
            y = 0.5 * np.sqrt(1 - R[0, 0] + R[1, 1] - R[2, 2])
            w = (R[0, 2] - R[2, 0]) / (4 * y)
            x = (R[0, 1] + R[1, 0]) / (4 * y)
            z = (R[1, 2] + R[2, 1]) / (4 * y)
        else:
            z = 0.5 * np.sqrt(1 - R[0, 0] - R[1, 1] + R[2, 2])
            w = (R[1, 0] - R[0, 1]) / (4 * z)
            x = (R[0, 2] + R[2, 0]) / (4 * z)
            y = (R[1, 2] + R[2, 1]) / (4 * z)
    return np.array([x, y, z, w])


def _reduce_half_pi(b):
    """reduce half-angle bias mod pi into [-pi/2, pi/2] (quat sign flip is
    absorbed by the output canonicalization)."""
    return float(b - PI * np.round(b / PI))


def _precompute(rot_fixed, trans_fixed):
    """Per-link constants from fp32 inputs (math in float64)."""
    rot = np.asarray(rot_fixed, dtype=np.float64)
    tf = np.asarray(trans_fixed, dtype=np.float64)
    links = []
    for i in range(N_LINKS):
        R = rot[i]
        if AXIS_IS_Z[i]:
            a, b, g = _euler_yxz(R)
            chk = _ry(a) @ _rx(b) @ _rz(g)
        else:
            a, b, g = _euler_zxy(R)
            chk = _rz(a) @ _rx(b) @ _ry(g)
        assert np.abs(chk - R).max() < 1e-5, (i, np.abs(chk - R).max())
        links.append(dict(
            alpha=a, beta=b,
            # var-rot half-angle bias, reduced into [-pi/2, pi/2]
            hbias=_reduce_half_pi(g / 2.0),
            ca=float(np.cos(a / 2)), sa=float(np.sin(a / 2)),
            cb=float(np.cos(b / 2)), sb=float(np.sin(b / 2)),
            tf=[float(v) for v in tf[i]],
        ))
    # link-1 translation via double angle: Delta_1 = cos(th)*A + sin(th)*B + C,
    # th = q_0 + 2*hbias_0;  e = Rz(-2*hbias_0) @ tf_1
    b0 = links[0]['hbias']
    e = _rz(-2.0 * b0) @ tf[1]
    Rf0 = rot[0]
    A1v = Rf0 @ np.array([e[0], e[1], 0.0])
    B1v = Rf0 @ np.array([-e[1], e[0], 0.0])
    C1v = Rf0 @ np.array([0.0, 0.0, e[2]])
    links[1]['A'] = [float(v) for v in A1v]
    links[1]['B'] = [float(v) for v in B1v]
    links[1]['K'] = [float(v) for v in (tf[0] + C1v + A1v)]
    # link 0 init quat: U = quat(Ry(a0) Rx(b0)), V = U (x) zhat
    U = _quat_from_R(_ry(links[0]['alpha']) @ _rx(links[0]['beta']))
    V = np.array([U[1], -U[0], U[3], -U[2]])
    links[0]['U'] = [float(v) for v in U]
    links[0]['V'] = [float(v) for v in V]
    return links


# ------------------------------------------------------------- bass program
def _ap_with(ap, offset, dims):
    """Build a raw AP view: same tensor, explicit [step, count] dims
    (partition dim first), offsets in elements."""
    return dataclasses.replace(ap, offset=offset, ap=type(ap.ap)(dims))


def _build_bass(links):
    import concourse.bass as bass  # noqa: F401
    from concourse import bacc
    import concourse.tile as tile
    import concourse.mybir as mybir
    from concourse.alu_op_type import AluOpType as Op

    dt = mybir.dt.float32
    AF = mybir.ActivationFunctionType

    nc = bacc.Bacc(trn_type="TRN2", target_bir_lowering=False, debug=False)

    qt_d = nc.dram_tensor("qt", [N_LINKS, PER_CORE], dt, kind="ExternalInput")
    kc_d = nc.dram_tensor("kcols", [P, 36], dt, kind="ExternalInput")
    ts_d = nc.dram_tensor("ts", [N_LINKS, PER_CORE, 3], dt, kind="ExternalOutput")
    qu_d = nc.dram_tensor("quats", [N_LINKS, PER_CORE, 4], dt, kind="ExternalOutput")

    with tile.TileContext(nc) as tc:
        with (
            tc.tile_pool(name="io", bufs=1) as io_pool,
            tc.tile_pool(name="bulk", bufs=1) as bulk_pool,
            tc.tile_pool(name="trig", bufs=3) as trig_pool,
            tc.tile_pool(name="state", bufs=5) as st_pool,
            tc.tile_pool(name="scr", bufs=1) as scr_pool,
            tc.tile_pool(name="scrP", bufs=1) as scrP_pool,
            tc.tile_pool(name="scr2", bufs=2) as scr2_pool,
            tc.tile_pool(name="stage", bufs=2) as stage_pool,
            tc.tile_pool(name="tstage", bufs=3) as tstage_pool,
        ):
            kc = io_pool.tile([P, 36], dt)
            nc.sync.dma_start(kc[:], kc_d.ap())
            PIH = kc[:, 0:1]  # pi/2 column

            qt = bulk_pool.tile([P, N_LINKS, F], dt, tag="qt")
            nc.sync.dma_start(qt[:],
                              qt_d.ap().rearrange("j (p f) -> p j f", p=P))

            def block(t, k, n=1):
                return t[:, k:k + n, :]

            def pair(t, k0, step):
                base = t[:].offset
                return _ap_with(t[:], base + k0 * F,
                                [[t[:].ap[0][0], P], [step * F, 2], [1, F]])

            # ---------------- per-link trig tiles -------------------------
            # T tile per link: 4 blocks (c, s, ns, s)
            T_tiles = []
            for i in range(N_LINKS):
                h = scr2_pool.tile([P, F], dt, tag="h")
                nc.scalar.activation(h[:], qt[:, i, :], AF.Identity,
                                     bias=kc[:, 19 + i:20 + i], scale=0.5)
                g1 = scr2_pool.tile([P, F], dt, tag="g1")
                nc.vector.tensor_scalar(g1[:], h[:], 1.6, PI,
                                        op0=Op.is_gt, op1=Op.mult)
                nc.vector.tensor_tensor(h[:], h[:], g1[:], op=Op.subtract)
                g2 = scr2_pool.tile([P, F], dt, tag="g1")
                nc.vector.tensor_scalar(g2[:], h[:], -1.6, PI,
                                        op0=Op.is_lt, op1=Op.mult)
                nc.vector.tensor_tensor(h[:], h[:], g2[:], op=Op.add)
                T = trig_pool.tile([P, 4, F], dt, tag="T")
                nc.scalar.activation(T[:, 0, :], h[:], AF.Sin, bias=PIH,
                                     scale=1.0)
                nc.scalar.activation(T[:, 2, :], h[:], AF.Sin, scale=-1.0)
                if AXIS_IS_Z[i]:
                    nc.scalar.activation(T[:, 1, :], h[:], AF.Sin)
                else:
                    hdup = h[:].unsqueeze(1).broadcast_to((P, 2, F))
                    sdup = _ap_with(T[:], T[:].offset + F,
                                    [[T[:].ap[0][0], P], [2 * F, 2], [1, F]])
                    nc.scalar.activation(sdup, hdup, AF.Sin)
                T_tiles.append(T)

            def Tc(i):
                return T_tiles[i][:, 0, :]

            def Tsn(i):
                return T_tiles[i][:, 1, :]

            D = None
            tprev_view = None

            for i in range(N_LINKS):
                L = links[i]

                # ---------------- translation output ----------------------
                tstage = tstage_pool.tile([P, F, 3], dt, tag="ts")
                tsv = tstage[:].transpose([0, 2, 1])
                if i == 0:
                    for c in range(3):
                        nc.scalar.activation(tsv[:, c, :], qt[:, 0, :],
                                             AF.Identity,
                                             bias=kc[:, 27 + c:28 + c],
                                             scale=0.0)
                elif i == 1:
                    # Delta_1 + t_0 = K - 2A*sin(h0)^2 + 2B*sin(h0)cos(h0)
                    sq = scr2_pool.tile([P, F], dt, tag="sq")
                    nc.scalar.activation(sq[:], Tsn(0), AF.Square)
                    sc = scr2_pool.tile([P, F], dt, tag="sc")
                    nc.vector.tensor_tensor(sc[:], Tsn(0), Tc(0), op=Op.mult)
                    zl = scr_pool.tile([P, 3, F], dt, tag="zz")
                    for c in range(3):
                        nc.scalar.activation(zl[:, c, :], sq[:], AF.Identity,
                                             bias=kc[:, 30 + c:31 + c],
                                             scale=-2.0 * L['A'][c])
                        nc.vector.scalar_tensor_tensor(
                            tsv[:, c, :], sc[:], 2.0 * L['B'][c], zl[:, c, :],
                            op0=Op.mult, op1=Op.add)
                else:
                    v = L['tf']
                    u = block(D, 0, 3)
                    w = block(D, 3)
                    t1 = scr_pool.tile([P, 3, F], dt, tag="t1")
                    nc.scalar.mul(t1[:, 0, :], D[:, 1, :], v[2])
                    nc.scalar.mul(t1[:, 1, :], D[:, 2, :], v[0])
                    nc.scalar.mul(t1[:, 2, :], D[:, 0, :], v[1])
                    td = scr_pool.tile([P, F], dt, tag="td")
                    nc.scalar.mul(td[:], D[:, 0, :], v[0])
                    w2 = scr_pool.tile([P, F], dt, tag="w2")
                    nc.scalar.activation(w2[:], D[:, 3, :], AF.Square)
                    zz = scr_pool.tile([P, 3, F], dt, tag="zz")
                    for c in range(3):
                        nc.scalar.activation(zz[:, c, :], w2[:], AF.Identity,
                                             bias=kc[:, 1 + 3 * (i - 1) + c:
                                                     2 + 3 * (i - 1) + c],
                                             scale=2.0 * v[c])
                    t2 = scr_pool.tile([P, 3, F], dt, tag="t2")
                    nc.scalar.mul(t2[:, 0, :], D[:, 2, :], -v[1])
                    nc.scalar.mul(t2[:, 1, :], D[:, 0, :], -v[2])
                    nc.scalar.mul(t2[:, 2, :], D[:, 1, :], -v[0])
                    SS = scr_pool.tile([P, 3, F], dt, tag="SS")
                    nc.vector.tensor_tensor(SS[:], t1[:], t2[:], op=Op.add)
                    dd = scr_pool.tile([P, F], dt, tag="dd")
                    nc.vector.scalar_tensor_tensor(dd[:], D[:, 1, :], v[1],
                                                   td[:], op0=Op.mult, op1=Op.add)
                    nc.vector.scalar_tensor_tensor(dd[:], D[:, 2, :], v[2],
                                                   dd[:], op0=Op.mult, op1=Op.add)
                    P1 = scr_pool.tile([P, 3, F], dt, tag="t1")
                    wb3 = w.broadcast_to((P, 3, F))
                    nc.vector.tensor_tensor(P1[:], SS[:], wb3, op=Op.mult)
                    P2 = scr_pool.tile([P, 3, F], dt, tag="SS")
                    ddb3 = dd[:].unsqueeze(1).broadcast_to((P, 3, F))
                    nc.vector.tensor_tensor(P2[:], u, ddb3, op=Op.mult)
                    nc.vector.tensor_tensor(P1[:], P1[:], P2[:], op=Op.add)
                    nc.vector.tensor_tensor(zz[:], zz[:], tprev_view, op=Op.add)
                    nc.vector.scalar_tensor_tensor(tsv, P1[:], 2.0, zz[:],
                                                   op0=Op.mult, op1=Op.add)
                nc.sync.dma_start(
                    ts_d.ap()[i].rearrange("(p f) c -> p f c", p=P), tstage[:])
                tprev_view = tsv

                # ---------------- rotations -------------------------------
                if i == 0:
                    D2 = st_pool.tile([P, 4, F], dt, tag="D")
                    for k in range(4):
                        nc.vector.tensor_scalar(D2[:, k, :], Tc(0),
                                                L['U'][k], None, op0=Op.mult)
                        nc.vector.scalar_tensor_tensor(
                            D2[:, k, :], Tsn(0), L['V'][k], D2[:, k, :],
                            op0=Op.mult, op1=Op.add)
                    D = D2
                else:
                    D = _const_rot(nc, st_pool, dt, Op, D,
                                   'z' if AXIS_IS_Z[i - 1] else 'y',
                                   L['ca'], L['sa'], pair)
                    D = _const_rot(nc, st_pool, dt, Op, D, 'x',
                                   L['cb'], L['sb'], pair)
                    D = _var_rot(nc, st_pool, scrP_pool, dt, Op, D,
                                 T_tiles[i], 'z' if AXIS_IS_Z[i] else 'y')

                # ---------------- canonicalize + stage quat ---------------
                # copy w's sign bit onto xyz: out = xyz XOR (w & 0x80000000)
                it = mybir.dt.int32
                mask = scr2_pool.tile([P, F], it, tag="flip")
                nc.vector.tensor_scalar(mask[:], D[:, 3, :].bitcast(it),
                                        -2147483648, None, op0=Op.bitwise_and)
                qstage = stage_pool.tile([P, F, 4], dt, tag="qs")
                qsv = qstage[:].transpose([0, 2, 1])
                mb3 = mask[:].unsqueeze(1).broadcast_to((P, 3, F))
                nc.vector.tensor_tensor(qsv[:, 0:3, :].bitcast(it),
                                        D[:, 0:3, :].bitcast(it), mb3,
                                        op=Op.bitwise_xor)
                nc.scalar.activation(qsv[:, 3, :], D[:, 3, :], AF.Abs)
                nc.sync.dma_start(
                    qu_d.ap()[i].rearrange("(p f) c -> p f c", p=P), qstage[:])

    nc.compile()
    return nc


def _const_rot(nc, st_pool, dt, Op, D, axis, ch, sh, pair):
    """D' = ch*D + sh*(D (x) axis_hat); 3 fused ops."""
    CD = st_pool.tile([P, 4, F], dt, tag="D")
    nc.scalar.mul(CD[:], D[:], ch)
    if axis == 'x':
        # sig_x = (w, z, -y, -x)
        nc.vector.scalar_tensor_tensor(pair(CD, 0, 1), pair(D, 3, -1), sh,
                                       pair(CD, 0, 1), op0=Op.mult, op1=Op.add)
        nc.vector.scalar_tensor_tensor(pair(CD, 2, 1), pair(D, 1, -1), -sh,
                                       pair(CD, 2, 1), op0=Op.mult, op1=Op.add)
    elif axis == 'z':
        # sig_z = (y, -x, w, -z): +s on (x,z) from (y,w); -s on (y,w) from (x,z)
        nc.vector.scalar_tensor_tensor(pair(CD, 0, 2), pair(D, 1, 2), sh,
                                       pair(CD, 0, 2), op0=Op.mult, op1=Op.add)
        nc.vector.scalar_tensor_tensor(pair(CD, 1, 2), pair(D, 0, 2), -sh,
                                       pair(CD, 1, 2), op0=Op.mult, op1=Op.add)
    else:
        # sig_y = (-z, w, x, -y): x' -= s*z; y' += s*w; z' += s*x; w' -= s*y
        nc.vector.scalar_tensor_tensor(pair(CD, 1, 1), pair(D, 3, -3), sh,
                                       pair(CD, 1, 1), op0=Op.mult, op1=Op.add)
        nc.vector.scalar_tensor_tensor(pair(CD, 0, 3), pair(D, 2, -1), -sh,
                                       pair(CD, 0, 3), op0=Op.mult, op1=Op.add)
    return CD


def _var_rot(nc, st_pool, scrP_pool, dt, Op, D, T, axis):
    """D' = c*D + s*(D (x) axis_hat); T blocks (c, s, ns, s)."""
    Tap = T[:]
    tstride = Tap.ap[0][0]
    tbase = Tap.offset
    CD = st_pool.tile([P, 4, F], dt, tag="D")
    cb4 = T[:, 0:1, :].broadcast_to((P, 4, F))
    nc.vector.tensor_tensor(CD[:], D[:], cb4, op=Op.mult)
    Pt = scrP_pool.tile([P, 4, F], dt, tag="P")
    Dap = D[:]
    dstride = Dap.ap[0][0]
    out = _ap_with(Pt[:], Pt[:].offset,
                   [[Pt[:].ap[0][0], P], [2 * F, 2], [F, 2], [1, F]])
    if axis == 'z':
        fac = _ap_with(Tap, tbase + F, [[tstride, P], [0, 2], [F, 2], [1, F]])
        op = _ap_with(Dap, Dap.offset + F,
                      [[dstride, P], [2 * F, 2], [-F, 2], [1, F]])
    else:
        fac = _ap_with(Tap, tbase + 2 * F,
                       [[tstride, P], [F, 2], [-F, 2], [1, F]])
        op = _ap_with(Dap, Dap.offset + 2 * F,
                      [[dstride, P], [-2 * F, 2], [F, 2], [1, F]])
    nc.vector.tensor_tensor(out, fac, op, op=Op.mult)
    nc.vector.tensor_tensor(CD[:], CD[:], Pt[:], op=Op.add)
    return CD


# ------------------------------------------------------------------ public
def _get_program(rot_fixed, trans_fixed):
    key = (np.asarray(rot_fixed, np.float32).tobytes(),
           np.asarray(trans_fixed, np.float32).tobytes())
    if key not in _CACHE:
        links = _precompute(rot_fixed, trans_fixed)
        nc = _build_bass(links)
        kc = np.zeros((P, 36), np.float32)
        kc[:, 0] = PI / 2
        for i in range(1, N_LINKS):
            for c in range(3):
                kc[:, 1 + 3 * (i - 1) + c] = -links[i]['tf'][c]
        for i in range(N_LINKS):
            kc[:, 19 + i] = links[i]['hbias']
        kc[:, 26] = -1.0
        for c in range(3):
            kc[:, 27 + c] = links[0]['tf'][c]
        for c in range(3):
            kc[:, 30 + c] = links[1]['K'][c]
        _CACHE[key] = (nc, kc)
    return _CACHE[key]


def run(q, rot_fixed, trans_fixed, trace=False):
    from concourse.bass_utils import run_bass_kernel_spmd
    nc, kc = _get_program(rot_fixed, trans_fixed)
    q = np.asarray(q, np.float32)
    assert q.shape == (BATCH, N_LINKS), q.shape
    in_maps = []
    for c in range(N_CORES):
        qc = np.ascontiguousarray(q[c * PER_CORE:(c + 1) * PER_CORE].T)
        in_maps.append(dict(qt=qc, kcols=kc))
    res = run_bass_kernel_spmd(nc, in_maps, core_ids=list(range(N_CORES)),
                               trace=trace)
    ts = np.empty((N_LINKS, BATCH, 3), np.float32)
    qu = np.empty((N_LINKS, BATCH, 4), np.float32)
    for c in range(N_CORES):
        ts[:, c * PER_CORE:(c + 1) * PER_CORE] = res.results[c]["ts"]
        qu[:, c * PER_CORE:(c + 1) * PER_CORE] = res.results[c]["quats"]
    return (ts, qu), res


def kernel(q, rot_fixed, trans_fixed):
    (ts, qu), _ = run(q, rot_fixed, trans_fixed, trace=False)
    return ts, qu


# revision 18
# speedup vs baseline: 1.0265x; 1.0265x over previous
"""Trainium2 Bass kernel for DifferentiableRobotModel forward kinematics.

Math: the chain quat  chain_i = qf_0 (x) Z(q_0) (x) qf_1 (x) Y(q_1) ...  is
evaluated as a sweep of single-axis rotations.  Each fixed rotation qf_i is
Euler-decomposed (host, float64) as A1(a_i) X(b_i) A3(g_i) where A3 = joint
axis of link i (z even / y odd) and A1 = joint axis of link i-1, so the
whole chain is:  per link: [A1(a_i)] [X(b_i)] [A3(g_i + q_i)] — two
constant-angle rotations + one variable-angle rotation.  A single-axis
right-multiplication on the quat state D is  D' = c*D + s*(D (x) axis_hat),
where (D (x) axis_hat) is a signed permutation of D's components —
expressible as strided access patterns, so each rotation is 3-4 fused DVE
ops on [128, 4*512] tiles.

Translations:  t_i = t_{i-1} + R(chain_{i-1}) @ tf_i  via the quat-rotate
identity  R(D)v = 2w(u x v) + 2u(u.v) + (2w^2-1)v  (unit quat), with the
constant-scale multiplies and affine terms offloaded to the Scalar engine.

Outputs match the reference's (x,y,z,w) quats canonicalized to w >= 0.

Data parallel over 8 NeuronCores: core c owns batch slice [c*65536,(c+1)*65536),
65536 = 128 partitions x 512 free elements per core.
"""
import sys
import dataclasses

sys.path.insert(0, '/opt/trn_rl_repo')

import numpy as np

N_LINKS = 7
N_CORES = 8
BATCH = 524288
PER_CORE = BATCH // N_CORES          # 65536
P = 128
F = PER_CORE // P                    # 512
PI = float(np.pi)
# joint axes: z for even links, y for odd links
AXIS_IS_Z = [True, False, True, False, True, False, True]

_CACHE = {}


# ---------------------------------------------------------------- host math
def _rx(a):
    c, s = np.cos(a), np.sin(a)
    return np.array([[1, 0, 0], [0, c, -s], [0, s, c]], dtype=np.float64)


def _ry(a):
    c, s = np.cos(a), np.sin(a)
    return np.array([[c, 0, s], [0, 1, 0], [-s, 0, c]], dtype=np.float64)


def _rz(a):
    c, s = np.cos(a), np.sin(a)
    return np.array([[c, -s, 0], [s, c, 0], [0, 0, 1]], dtype=np.float64)


def _euler_yxz(R):
    # R = Ry(a) Rx(b) Rz(g)
    b = np.arcsin(np.clip(-R[1, 2], -1, 1))
    a = np.arctan2(R[0, 2], R[2, 2])
    g = np.arctan2(R[1, 0], R[1, 1])
    return a, b, g


def _euler_zxy(R):
    # R = Rz(a) Rx(b) Ry(g)
    b = np.arcsin(np.clip(R[2, 1], -1, 1))
    a = np.arctan2(-R[0, 1], R[1, 1])
    g = np.arctan2(-R[2, 0], R[2, 2])
    return a, b, g


def _quat_from_R(R):
    # float64 rotation -> quat (x, y, z, w), any sign
    t = np.trace(R)
    if t > 0:
        w = 0.5 * np.sqrt(1 + t)
        x = (R[2, 1] - R[1, 2]) / (4 * w)
        y = (R[0, 2] - R[2, 0]) / (4 * w)
        z = (R[1, 0] - R[0, 1]) / (4 * w)
    else:
        i = int(np.argmax(np.diag(R)))
        if i == 0:
            x = 0.5 * np.sqrt(1 + R[0, 0] - R[1, 1] - R[2, 2])
            w = (R[2, 1] - R[1, 2]) / (4 * x)
            y = (R[0, 1] + R[1, 0]) / (4 * x)
            z = (R[0, 2] + R[2, 0]) / (4 * x)
        elif i == 1:
            y = 0.5 * np.sqrt(1 - R[0, 0] + R[1, 1] - R[2, 2])
            w = (R[0, 2] - R[2, 0]) / (4 * y)
            x = (R[0, 1] + R[1, 0]) / (4 * y)
            z = (R[1, 2] + R[2, 1]) / (4 * y)
        else:
            z = 0.5 * np.sqrt(1 - R[0, 0] - R[1, 1] + R[2, 2])
            w = (R[1, 0] - R[0, 1]) / (4 * z)
            x = (R[0, 2] + R[2, 0]) / (4 * z)
            y = (R[1, 2] + R[2, 1]) / (4 * z)
    return np.array([x, y, z, w])


def _reduce_half_pi(b):
    """reduce half-angle bias mod pi into [-pi/2, pi/2] (quat sign flip is
    absorbed by the output canonicalization)."""
    return float(b - PI * np.round(b / PI))


def _precompute(rot_fixed, trans_fixed):
    """Per-link constants from fp32 inputs (math in float64)."""
    rot = np.asarray(rot_fixed, dtype=np.float64)
    tf = np.asarray(trans_fixed, dtype=np.float64)
    links = []
    for i in range(N_LINKS):
        R = rot[i]
        if AXIS_IS_Z[i]:
            a, b, g = _euler_yxz(R)
            chk = _ry(a) @ _rx(b) @ _rz(g)
        else:
            a, b, g = _euler_zxy(R)
            chk = _rz(a) @ _rx(b) @ _ry(g)
        assert np.abs(chk - R).max() < 1e-5, (i, np.abs(chk - R).max())
        links.append(dict(
            alpha=a, beta=b,
            # var-rot half-angle bias, reduced into [-pi/2, pi/2]
            hbias=_reduce_half_pi(g / 2.0),
            ca=float(np.cos(a / 2)), sa=float(np.sin(a / 2)),
            cb=float(np.cos(b / 2)), sb=float(np.sin(b / 2)),
            tf=[float(v) for v in tf[i]],
        ))
    # link-1 translation via double angle: Delta_1 = cos(th)*A + sin(th)*B + C,
    # th = q_0 + 2*hbias_0;  e = Rz(-2*hbias_0) @ tf_1
    b0 = links[0]['hbias']
    e = _rz(-2.0 * b0) @ tf[1]
    Rf0 = rot[0]
    A1v = Rf0 @ np.array([e[0], e[1], 0.0])
    B1v = Rf0 @ np.array([-e[1], e[0], 0.0])
    C1v = Rf0 @ np.array([0.0, 0.0, e[2]])
    links[1]['A'] = [float(v) for v in A1v]
    links[1]['B'] = [float(v) for v in B1v]
    links[1]['K'] = [float(v) for v in (tf[0] + C1v + A1v)]
    # link 0 init quat: U = quat(Ry(a0) Rx(b0)), V = U (x) zhat
    U = _quat_from_R(_ry(links[0]['alpha']) @ _rx(links[0]['beta']))
    V = np.array([U[1], -U[0], U[3], -U[2]])
    links[0]['U'] = [float(v) for v in U]
    links[0]['V'] = [float(v) for v in V]
    return links


# ------------------------------------------------------------- bass program
def _ap_with(ap, offset, dims):
    """Build a raw AP view: same tensor, explicit [step, count] dims
    (partition dim first), offsets in elements."""
    return dataclasses.replace(ap, offset=offset, ap=type(ap.ap)(dims))


def _build_bass(links):
    import concourse.bass as bass  # noqa: F401
    from concourse import bacc
    import concourse.tile as tile
    import concourse.mybir as mybir
    from concourse.alu_op_type import AluOpType as Op

    dt = mybir.dt.float32
    AF = mybir.ActivationFunctionType

    nc = bacc.Bacc(trn_type="TRN2", target_bir_lowering=False, debug=False)

    qt_d = nc.dram_tensor("qt", [N_LINKS, PER_CORE], dt, kind="ExternalInput")
    kc_d = nc.dram_tensor("kcols", [P, 36], dt, kind="ExternalInput")
    ts_d = nc.dram_tensor("ts", [N_LINKS, PER_CORE, 3], dt, kind="ExternalOutput")
    qu_d = nc.dram_tensor("quats", [N_LINKS, PER_CORE, 4], dt, kind="ExternalOutput")

    with tile.TileContext(nc) as tc:
        with (
            tc.tile_pool(name="io", bufs=1) as io_pool,
            tc.tile_pool(name="bulk", bufs=1) as bulk_pool,
            tc.tile_pool(name="trig", bufs=3) as trig_pool,
            tc.tile_pool(name="state", bufs=5) as st_pool,
            tc.tile_pool(name="scr", bufs=1) as scr_pool,
            tc.tile_pool(name="scrP", bufs=1) as scrP_pool,
            tc.tile_pool(name="scr2", bufs=2) as scr2_pool,
            tc.tile_pool(name="stage", bufs=2) as stage_pool,
            tc.tile_pool(name="tstage", bufs=3) as tstage_pool,
        ):
            kc = io_pool.tile([P, 36], dt)
            nc.sync.dma_start(kc[:], kc_d.ap())
            PIH = kc[:, 0:1]  # pi/2 column

            qt = bulk_pool.tile([P, N_LINKS, F], dt, tag="qt")
            for i in range(N_LINKS):
                nc.sync.dma_start(
                    qt[:, i, :], qt_d.ap()[i].rearrange("(p f) -> p f", p=P))

            def block(t, k, n=1):
                return t[:, k:k + n, :]

            def pair(t, k0, step):
                base = t[:].offset
                return _ap_with(t[:], base + k0 * F,
                                [[t[:].ap[0][0], P], [step * F, 2], [1, F]])

            # ---------------- per-link trig tiles -------------------------
            # T tile per link: 4 blocks (c, s, ns, s)
            T_tiles = []
            for i in range(N_LINKS):
                h = scr2_pool.tile([P, F], dt, tag="h")
                nc.scalar.activation(h[:], qt[:, i, :], AF.Identity,
                                     bias=kc[:, 19 + i:20 + i], scale=0.5)
                g1 = scr2_pool.tile([P, F], dt, tag="g1")
                nc.vector.tensor_scalar(g1[:], h[:], 1.6, PI,
                                        op0=Op.is_gt, op1=Op.mult)
                nc.vector.tensor_tensor(h[:], h[:], g1[:], op=Op.subtract)
                g2 = scr2_pool.tile([P, F], dt, tag="g1")
                nc.vector.tensor_scalar(g2[:], h[:], -1.6, PI,
                                        op0=Op.is_lt, op1=Op.mult)
                nc.vector.tensor_tensor(h[:], h[:], g2[:], op=Op.add)
                T = trig_pool.tile([P, 4, F], dt, tag="T")
                nc.scalar.activation(T[:, 0, :], h[:], AF.Sin, bias=PIH,
                                     scale=1.0)
                nc.scalar.activation(T[:, 2, :], h[:], AF.Sin, scale=-1.0)
                hdup = h[:].unsqueeze(1).broadcast_to((P, 2, F))
                sdup = _ap_with(T[:], T[:].offset + F,
                                [[T[:].ap[0][0], P], [2 * F, 2], [1, F]])
                nc.scalar.activation(sdup, hdup, AF.Sin)
                T_tiles.append(T)

            def Tc(i):
                return T_tiles[i][:, 0, :]

            def Tsn(i):
                return T_tiles[i][:, 1, :]

            D = None
            tprev_view = None

            for i in range(N_LINKS):
                L = links[i]

                # ---------------- translation output ----------------------
                tstage = tstage_pool.tile([P, F, 3], dt, tag="ts")
                tsv = tstage[:].transpose([0, 2, 1])
                if i == 0:
                    for c in range(3):
                        nc.scalar.activation(tsv[:, c, :], qt[:, 0, :],
                                             AF.Identity,
                                             bias=kc[:, 27 + c:28 + c],
                                             scale=0.0)
                elif i == 1:
                    # Delta_1 + t_0 = K - 2A*sin(h0)^2 + 2B*sin(h0)cos(h0)
                    sq = scr2_pool.tile([P, F], dt, tag="sq")
                    nc.scalar.activation(sq[:], Tsn(0), AF.Square)
                    sc = scr2_pool.tile([P, F], dt, tag="sc")
                    nc.vector.tensor_tensor(sc[:], Tsn(0), Tc(0), op=Op.mult)
                    zl = scr_pool.tile([P, 3, F], dt, tag="zz")
                    for c in range(3):
                        nc.scalar.activation(zl[:, c, :], sq[:], AF.Identity,
                                             bias=kc[:, 30 + c:31 + c],
                                             scale=-2.0 * L['A'][c])
                        nc.vector.scalar_tensor_tensor(
                            tsv[:, c, :], sc[:], 2.0 * L['B'][c], zl[:, c, :],
                            op0=Op.mult, op1=Op.add)
                else:
                    v = L['tf']
                    u = block(D, 0, 3)
                    w = block(D, 3)
                    t1 = scr_pool.tile([P, 3, F], dt, tag="t1")
                    nc.scalar.mul(t1[:, 0, :], D[:, 1, :], v[2])
                    nc.scalar.mul(t1[:, 1, :], D[:, 2, :], v[0])
                    nc.scalar.mul(t1[:, 2, :], D[:, 0, :], v[1])
                    td = scr_pool.tile([P, F], dt, tag="td")
                    nc.scalar.mul(td[:], D[:, 0, :], v[0])
                    w2 = scr_pool.tile([P, F], dt, tag="w2")
                    nc.scalar.activation(w2[:], D[:, 3, :], AF.Square)
                    zz = scr_pool.tile([P, 3, F], dt, tag="zz")
                    for c in range(3):
                        nc.scalar.activation(zz[:, c, :], w2[:], AF.Identity,
                                             bias=kc[:, 1 + 3 * (i - 1) + c:
                                                     2 + 3 * (i - 1) + c],
                                             scale=2.0 * v[c])
                    t2 = scr_pool.tile([P, 3, F], dt, tag="t2")
                    nc.scalar.mul(t2[:, 0, :], D[:, 2, :], -v[1])
                    nc.scalar.mul(t2[:, 1, :], D[:, 0, :], -v[2])
                    nc.scalar.mul(t2[:, 2, :], D[:, 1, :], -v[0])
                    SS = scr_pool.tile([P, 3, F], dt, tag="SS")
                    nc.vector.tensor_tensor(SS[:], t1[:], t2[:], op=Op.add)
                    dd = scr_pool.tile([P, F], dt, tag="dd")
                    nc.vector.scalar_tensor_tensor(dd[:], D[:, 1, :], v[1],
                                                   td[:], op0=Op.mult, op1=Op.add)
                    nc.vector.scalar_tensor_tensor(dd[:], D[:, 2, :], v[2],
                                                   dd[:], op0=Op.mult, op1=Op.add)
                    P1 = scr_pool.tile([P, 3, F], dt, tag="t1")
                    wb3 = w.broadcast_to((P, 3, F))
                    nc.vector.tensor_tensor(P1[:], SS[:], wb3, op=Op.mult)
                    P2 = scr_pool.tile([P, 3, F], dt, tag="SS")
                    ddb3 = dd[:].unsqueeze(1).broadcast_to((P, 3, F))
                    nc.vector.tensor_tensor(P2[:], u, ddb3, op=Op.mult)
                    nc.vector.tensor_tensor(P1[:], P1[:], P2[:], op=Op.add)
                    nc.vector.tensor_tensor(zz[:], zz[:], tprev_view, op=Op.add)
                    nc.vector.scalar_tensor_tensor(tsv, P1[:], 2.0, zz[:],
                                                   op0=Op.mult, op1=Op.add)
                nc.sync.dma_start(
                    ts_d.ap()[i].rearrange("(p f) c -> p f c", p=P), tstage[:])
                tprev_view = tsv

                # ---------------- rotations -------------------------------
                if i == 0:
                    D2 = st_pool.tile([P, 4, F], dt, tag="D")
                    for k in range(4):
                        nc.vector.tensor_scalar(D2[:, k, :], Tc(0),
                                                L['U'][k], None, op0=Op.mult)
                        nc.vector.scalar_tensor_tensor(
                            D2[:, k, :], Tsn(0), L['V'][k], D2[:, k, :],
                            op0=Op.mult, op1=Op.add)
                    D = D2
                else:
                    D = _const_rot(nc, st_pool, dt, Op, D,
                                   'z' if AXIS_IS_Z[i - 1] else 'y',
                                   L['ca'], L['sa'], pair)
                    D = _const_rot(nc, st_pool, dt, Op, D, 'x',
                                   L['cb'], L['sb'], pair)
                    D = _var_rot(nc, st_pool, scrP_pool, dt, Op, D,
                                 T_tiles[i], 'z' if AXIS_IS_Z[i] else 'y')

                # ---------------- canonicalize + stage quat ---------------
                # copy w's sign bit onto xyz: out = xyz XOR (w & 0x80000000)
                it = mybir.dt.int32
                mask = scr2_pool.tile([P, F], it, tag="flip")
                nc.vector.tensor_scalar(mask[:], D[:, 3, :].bitcast(it),
                                        -2147483648, None, op0=Op.bitwise_and)
                qstage = stage_pool.tile([P, F, 4], dt, tag="qs")
                qsv = qstage[:].transpose([0, 2, 1])
                mb3 = mask[:].unsqueeze(1).broadcast_to((P, 3, F))
                nc.vector.tensor_tensor(qsv[:, 0:3, :].bitcast(it),
                                        D[:, 0:3, :].bitcast(it), mb3,
                                        op=Op.bitwise_xor)
                nc.scalar.activation(qsv[:, 3, :], D[:, 3, :], AF.Abs)
                nc.sync.dma_start(
                    qu_d.ap()[i].rearrange("(p f) c -> p f c", p=P), qstage[:])

    nc.compile()
    return nc


def _const_rot(nc, st_pool, dt, Op, D, axis, ch, sh, pair):
    """D' = ch*D + sh*(D (x) axis_hat); 3 fused ops."""
    CD = st_pool.tile([P, 4, F], dt, tag="D")
    nc.scalar.mul(CD[:], D[:], ch)
    if axis == 'x':
        # sig_x = (w, z, -y, -x)
        nc.vector.scalar_tensor_tensor(pair(CD, 0, 1), pair(D, 3, -1), sh,
                                       pair(CD, 0, 1), op0=Op.mult, op1=Op.add)
        nc.vector.scalar_tensor_tensor(pair(CD, 2, 1), pair(D, 1, -1), -sh,
                                       pair(CD, 2, 1), op0=Op.mult, op1=Op.add)
    elif axis == 'z':
        # sig_z = (y, -x, w, -z): +s on (x,z) from (y,w); -s on (y,w) from (x,z)
        nc.vector.scalar_tensor_tensor(pair(CD, 0, 2), pair(D, 1, 2), sh,
                                       pair(CD, 0, 2), op0=Op.mult, op1=Op.add)
        nc.vector.scalar_tensor_tensor(pair(CD, 1, 2), pair(D, 0, 2), -sh,
                                       pair(CD, 1, 2), op0=Op.mult, op1=Op.add)
    else:
        # sig_y = (-z, w, x, -y): x' -= s*z; y' += s*w; z' += s*x; w' -= s*y
        nc.vector.scalar_tensor_tensor(pair(CD, 1, 1), pair(D, 3, -3), sh,
                                       pair(CD, 1, 1), op0=Op.mult, op1=Op.add)
        nc.vector.scalar_tensor_tensor(pair(CD, 0, 3), pair(D, 2, -1), -sh,
                                       pair(CD, 0, 3), op0=Op.mult, op1=Op.add)
    return CD


def _var_rot(nc, st_pool, scrP_pool, dt, Op, D, T, axis):
    """D' = c*D + s*(D (x) axis_hat); T blocks (c, s, ns, s)."""
    Tap = T[:]
    tstride = Tap.ap[0][0]
    tbase = Tap.offset
    CD = st_pool.tile([P, 4, F], dt, tag="D")
    cb4 = T[:, 0:1, :].broadcast_to((P, 4, F))
    nc.vector.tensor_tensor(CD[:], D[:], cb4, op=Op.mult)
    Pt = scrP_pool.tile([P, 4, F], dt, tag="P")
    Dap = D[:]
    dstride = Dap.ap[0][0]
    out = _ap_with(Pt[:], Pt[:].offset,
                   [[Pt[:].ap[0][0], P], [2 * F, 2], [F, 2], [1, F]])
    if axis == 'z':
        fac = _ap_with(Tap, tbase + F, [[tstride, P], [0, 2], [F, 2], [1, F]])
        op = _ap_with(Dap, Dap.offset + F,
                      [[dstride, P], [2 * F, 2], [-F, 2], [1, F]])
    else:
        fac = _ap_with(Tap, tbase + 2 * F,
                       [[tstride, P], [F, 2], [-F, 2], [1, F]])
        op = _ap_with(Dap, Dap.offset + 2 * F,
                      [[dstride, P], [-2 * F, 2], [F, 2], [1, F]])
    nc.vector.tensor_tensor(out, fac, op, op=Op.mult)
    nc.vector.tensor_tensor(CD[:], CD[:], Pt[:], op=Op.add)
    return CD


# ------------------------------------------------------------------ public
def _get_program(rot_fixed, trans_fixed):
    key = (np.asarray(rot_fixed, np.float32).tobytes(),
           np.asarray(trans_fixed, np.float32).tobytes())
    if key not in _CACHE:
        links = _precompute(rot_fixed, trans_fixed)
        nc = _build_bass(links)
        kc = np.zeros((P, 36), np.float32)
        kc[:, 0] = PI / 2
        for i in range(1, N_LINKS):
            for c in range(3):
                kc[:, 1 + 3 * (i - 1) + c] = -links[i]['tf'][c]
        for i in range(N_LINKS):
            kc[:, 19 + i] = links[i]['hbias']
        kc[:, 26] = -1.0
        for c in range(3):
            kc[:, 27 + c] = links[0]['tf'][c]
        for c in range(3):
            kc[:, 30 + c] = links[1]['K'][c]
        _CACHE[key] = (nc, kc)
    return _CACHE[key]


def run(q, rot_fixed, trans_fixed, trace=False):
    from concourse.bass_utils import run_bass_kernel_spmd
    nc, kc = _get_program(rot_fixed, trans_fixed)
    q = np.asarray(q, np.float32)
    assert q.shape == (BATCH, N_LINKS), q.shape
    in_maps = []
    for c in range(N_CORES):
        qc = np.ascontiguousarray(q[c * PER_CORE:(c + 1) * PER_CORE].T)
        in_maps.append(dict(qt=qc, kcols=kc))
    res = run_bass_kernel_spmd(nc, in_maps, core_ids=list(range(N_CORES)),
                               trace=trace)
    ts = np.empty((N_LINKS, BATCH, 3), np.float32)
    qu = np.empty((N_LINKS, BATCH, 4), np.float32)
    for c in range(N_CORES):
        ts[:, c * PER_CORE:(c + 1) * PER_CORE] = res.results[c]["ts"]
        qu[:, c * PER_CORE:(c + 1) * PER_CORE] = res.results[c]["quats"]
    return (ts, qu), res


def kernel(q, rot_fixed, trans_fixed):
    (ts, qu), _ = run(q, rot_fixed, trans_fixed, trace=False)
    return ts, qu


# revision 19
# speedup vs baseline: 1.0324x; 1.0058x over previous
"""Trainium2 Bass kernel for DifferentiableRobotModel forward kinematics.

Math: the chain quat  chain_i = qf_0 (x) Z(q_0) (x) qf_1 (x) Y(q_1) ...  is
evaluated as a sweep of single-axis rotations.  Each fixed rotation qf_i is
Euler-decomposed (host, float64) as A1(a_i) X(b_i) A3(g_i) where A3 = joint
axis of link i (z even / y odd) and A1 = joint axis of link i-1, so the
whole chain is:  per link: [A1(a_i)] [X(b_i)] [A3(g_i + q_i)] — two
constant-angle rotations + one variable-angle rotation.  A single-axis
right-multiplication on the quat state D is  D' = c*D + s*(D (x) axis_hat),
where (D (x) axis_hat) is a signed permutation of D's components —
expressible as strided access patterns, so each rotation is 3-4 fused DVE
ops on [128, 4*512] tiles.

Translations:  t_i = t_{i-1} + R(chain_{i-1}) @ tf_i  via the quat-rotate
identity  R(D)v = 2w(u x v) + 2u(u.v) + (2w^2-1)v  (unit quat), with the
constant-scale multiplies and affine terms offloaded to the Scalar engine.

Outputs match the reference's (x,y,z,w) quats canonicalized to w >= 0.

Data parallel over 8 NeuronCores: core c owns batch slice [c*65536,(c+1)*65536),
65536 = 128 partitions x 512 free elements per core.
"""
import sys
import dataclasses

sys.path.insert(0, '/opt/trn_rl_repo')

import numpy as np

N_LINKS = 7
N_CORES = 8
BATCH = 524288
PER_CORE = BATCH // N_CORES          # 65536
P = 128
F = PER_CORE // P                    # 512
PI = float(np.pi)
# joint axes: z for even links, y for odd links
AXIS_IS_Z = [True, False, True, False, True, False, True]

_CACHE = {}


# ---------------------------------------------------------------- host math
def _rx(a):
    c, s = np.cos(a), np.sin(a)
    return np.array([[1, 0, 0], [0, c, -s], [0, s, c]], dtype=np.float64)


def _ry(a):
    c, s = np.cos(a), np.sin(a)
    return np.array([[c, 0, s], [0, 1, 0], [-s, 0, c]], dtype=np.float64)


def _rz(a):
    c, s = np.cos(a), np.sin(a)
    return np.array([[c, -s, 0], [s, c, 0], [0, 0, 1]], dtype=np.float64)


def _euler_yxz(R):
    # R = Ry(a) Rx(b) Rz(g)
    b = np.arcsin(np.clip(-R[1, 2], -1, 1))
    a = np.arctan2(R[0, 2], R[2, 2])
    g = np.arctan2(R[1, 0], R[1, 1])
    return a, b, g


def _euler_zxy(R):
    # R = Rz(a) Rx(b) Ry(g)
    b = np.arcsin(np.clip(R[2, 1], -1, 1))
    a = np.arctan2(-R[0, 1], R[1, 1])
    g = np.arctan2(-R[2, 0], R[2, 2])
    return a, b, g


def _quat_from_R(R):
    # float64 rotation -> quat (x, y, z, w), any sign
    t = np.trace(R)
    if t > 0:
        w = 0.5 * np.sqrt(1 + t)
        x = (R[2, 1] - R[1, 2]) / (4 * w)
        y = (R[0, 2] - R[2, 0]) / (4 * w)
        z = (R[1, 0] - R[0, 1]) / (4 * w)
    else:
        i = int(np.argmax(np.diag(R)))
        if i == 0:
            x = 0.5 * np.sqrt(1 + R[0, 0] - R[1, 1] - R[2, 2])
            w = (R[2, 1] - R[1, 2]) / (4 * x)
            y = (R[0, 1] + R[1, 0]) / (4 * x)
            z = (R[0, 2] + R[2, 0]) / (4 * x)
        elif i == 1:
            y = 0.5 * np.sqrt(1 - R[0, 0] + R[1, 1] - R[2, 2])
            w = (R[0, 2] - R[2, 0]) / (4 * y)
            x = (R[0, 1] + R[1, 0]) / (4 * y)
            z = (R[1, 2] + R[2, 1]) / (4 * y)
        else:
            z = 0.5 * np.sqrt(1 - R[0, 0] - R[1, 1] + R[2, 2])
            w = (R[1, 0] - R[0, 1]) / (4 * z)
            x = (R[0, 2] + R[2, 0]) / (4 * z)
            y = (R[1, 2] + R[2, 1]) / (4 * z)
    return np.array([x, y, z, w])


def _reduce_half_pi(b):
    """reduce half-angle bias mod pi into [-pi/2, pi/2] (quat sign flip is
    absorbed by the output canonicalization)."""
    return float(b - PI * np.round(b / PI))


def _precompute(rot_fixed, trans_fixed):
    """Per-link constants from fp32 inputs (math in float64)."""
    rot = np.asarray(rot_fixed, dtype=np.float64)
    tf = np.asarray(trans_fixed, dtype=np.float64)
    links = []
    for i in range(N_LINKS):
        R = rot[i]
        if AXIS_IS_Z[i]:
            a, b, g = _euler_yxz(R)
            chk = _ry(a) @ _rx(b) @ _rz(g)
        else:
            a, b, g = _euler_zxy(R)
            chk = _rz(a) @ _rx(b) @ _ry(g)
        assert np.abs(chk - R).max() < 1e-5, (i, np.abs(chk - R).max())
        links.append(dict(
            alpha=a, beta=b,
            # var-rot half-angle bias, reduced into [-pi/2, pi/2]
            hbias=_reduce_half_pi(g / 2.0),
            ca=float(np.cos(a / 2)), sa=float(np.sin(a / 2)),
            cb=float(np.cos(b / 2)), sb=float(np.sin(b / 2)),
            tf=[float(v) for v in tf[i]],
        ))
    # link-1 translation via double angle: Delta_1 = cos(th)*A + sin(th)*B + C,
    # th = q_0 + 2*hbias_0;  e = Rz(-2*hbias_0) @ tf_1
    b0 = links[0]['hbias']
    e = _rz(-2.0 * b0) @ tf[1]
    Rf0 = rot[0]
    A1v = Rf0 @ np.array([e[0], e[1], 0.0])
    B1v = Rf0 @ np.array([-e[1], e[0], 0.0])
    C1v = Rf0 @ np.array([0.0, 0.0, e[2]])
    links[1]['A'] = [float(v) for v in A1v]
    links[1]['B'] = [float(v) for v in B1v]
    links[1]['K'] = [float(v) for v in (tf[0] + C1v + A1v)]
    # link 0 init quat: U = quat(Ry(a0) Rx(b0)), V = U (x) zhat
    U = _quat_from_R(_ry(links[0]['alpha']) @ _rx(links[0]['beta']))
    V = np.array([U[1], -U[0], U[3], -U[2]])
    links[0]['U'] = [float(v) for v in U]
    links[0]['V'] = [float(v) for v in V]
    return links


# ------------------------------------------------------------- bass program
def _ap_with(ap, offset, dims):
    """Build a raw AP view: same tensor, explicit [step, count] dims
    (partition dim first), offsets in elements."""
    return dataclasses.replace(ap, offset=offset, ap=type(ap.ap)(dims))


def _build_bass(links):
    import concourse.bass as bass  # noqa: F401
    from concourse import bacc
    import concourse.tile as tile
    import concourse.mybir as mybir
    from concourse.alu_op_type import AluOpType as Op

    dt = mybir.dt.float32
    AF = mybir.ActivationFunctionType

    nc = bacc.Bacc(trn_type="TRN2", target_bir_lowering=False, debug=False)

    qt_d = nc.dram_tensor("qt", [N_LINKS, PER_CORE], dt, kind="ExternalInput")
    kc_d = nc.dram_tensor("kcols", [P, 36], dt, kind="ExternalInput")
    ts_d = nc.dram_tensor("ts", [N_LINKS, PER_CORE, 3], dt, kind="ExternalOutput")
    qu_d = nc.dram_tensor("quats", [N_LINKS, PER_CORE, 4], dt, kind="ExternalOutput")

    with tile.TileContext(nc, pool_alloc_mode="queue") as tc:
        with (
            tc.tile_pool(name="io", bufs=1) as io_pool,
            tc.tile_pool(name="bulk", bufs=1) as bulk_pool,
            tc.tile_pool(name="trig", bufs=3) as trig_pool,
            tc.tile_pool(name="state", bufs=5) as st_pool,
            tc.tile_pool(name="scr", bufs=1) as scr_pool,
            tc.tile_pool(name="scrP", bufs=1) as scrP_pool,
            tc.tile_pool(name="scr2", bufs=2) as scr2_pool,
            tc.tile_pool(name="stage", bufs=2) as stage_pool,
            tc.tile_pool(name="tstage", bufs=3) as tstage_pool,
        ):
            kc = io_pool.tile([P, 36], dt)
            nc.sync.dma_start(kc[:], kc_d.ap())
            PIH = kc[:, 0:1]  # pi/2 column

            qt = bulk_pool.tile([P, N_LINKS, F], dt, tag="qt")
            for i in range(N_LINKS):
                nc.sync.dma_start(
                    qt[:, i, :], qt_d.ap()[i].rearrange("(p f) -> p f", p=P))

            def block(t, k, n=1):
                return t[:, k:k + n, :]

            def pair(t, k0, step):
                base = t[:].offset
                return _ap_with(t[:], base + k0 * F,
                                [[t[:].ap[0][0], P], [step * F, 2], [1, F]])

            # ---------------- per-link trig tiles -------------------------
            # T tile per link: 4 blocks (c, s, ns, s)
            T_tiles = []
            for i in range(N_LINKS):
                h = scr2_pool.tile([P, F], dt, tag="h")
                nc.scalar.activation(h[:], qt[:, i, :], AF.Identity,
                                     bias=kc[:, 19 + i:20 + i], scale=0.5)
                g1 = scr2_pool.tile([P, F], dt, tag="g1")
                nc.vector.tensor_scalar(g1[:], h[:], 1.6, PI,
                                        op0=Op.is_gt, op1=Op.mult)
                nc.vector.tensor_tensor(h[:], h[:], g1[:], op=Op.subtract)
                g2 = scr2_pool.tile([P, F], dt, tag="g1")
                nc.vector.tensor_scalar(g2[:], h[:], -1.6, PI,
                                        op0=Op.is_lt, op1=Op.mult)
                nc.vector.tensor_tensor(h[:], h[:], g2[:], op=Op.add)
                T = trig_pool.tile([P, 4, F], dt, tag="T")
                nc.scalar.activation(T[:, 0, :], h[:], AF.Sin, bias=PIH,
                                     scale=1.0)
                nc.scalar.activation(T[:, 2, :], h[:], AF.Sin, scale=-1.0)
                hdup = h[:].unsqueeze(1).broadcast_to((P, 2, F))
                sdup = _ap_with(T[:], T[:].offset + F,
                                [[T[:].ap[0][0], P], [2 * F, 2], [1, F]])
                nc.scalar.activation(sdup, hdup, AF.Sin)
                T_tiles.append(T)

            def Tc(i):
                return T_tiles[i][:, 0, :]

            def Tsn(i):
                return T_tiles[i][:, 1, :]

            D = None
            tprev_view = None

            for i in range(N_LINKS):
                L = links[i]

                # ---------------- translation output ----------------------
                tstage = tstage_pool.tile([P, F, 3], dt, tag="ts")
                tsv = tstage[:].transpose([0, 2, 1])
                if i == 0:
                    for c in range(3):
                        nc.scalar.activation(tsv[:, c, :], qt[:, 0, :],
                                             AF.Identity,
                                             bias=kc[:, 27 + c:28 + c],
                                             scale=0.0)
                elif i == 1:
                    # Delta_1 + t_0 = K - 2A*sin(h0)^2 + 2B*sin(h0)cos(h0)
                    sq = scr2_pool.tile([P, F], dt, tag="sq")
                    nc.scalar.activation(sq[:], Tsn(0), AF.Square)
                    sc = scr2_pool.tile([P, F], dt, tag="sc")
                    nc.vector.tensor_tensor(sc[:], Tsn(0), Tc(0), op=Op.mult)
                    zl = scr_pool.tile([P, 3, F], dt, tag="zz")
                    for c in range(3):
                        nc.scalar.activation(zl[:, c, :], sq[:], AF.Identity,
                                             bias=kc[:, 30 + c:31 + c],
                                             scale=-2.0 * L['A'][c])
                        nc.vector.scalar_tensor_tensor(
                            tsv[:, c, :], sc[:], 2.0 * L['B'][c], zl[:, c, :],
                            op0=Op.mult, op1=Op.add)
                else:
                    v = L['tf']
                    u = block(D, 0, 3)
                    w = block(D, 3)
                    t1 = scr_pool.tile([P, 3, F], dt, tag="t1")
                    nc.scalar.mul(t1[:, 0, :], D[:, 1, :], v[2])
                    nc.scalar.mul(t1[:, 1, :], D[:, 2, :], v[0])
                    nc.scalar.mul(t1[:, 2, :], D[:, 0, :], v[1])
                    td = scr_pool.tile([P, F], dt, tag="td")
                    nc.scalar.mul(td[:], D[:, 0, :], v[0])
                    w2 = scr_pool.tile([P, F], dt, tag="w2")
                    nc.scalar.activation(w2[:], D[:, 3, :], AF.Square)
                    zz = scr_pool.tile([P, 3, F], dt, tag="zz")
                    for c in range(3):
                        nc.scalar.activation(zz[:, c, :], w2[:], AF.Identity,
                                             bias=kc[:, 1 + 3 * (i - 1) + c:
                                                     2 + 3 * (i - 1) + c],
                                             scale=2.0 * v[c])
                    t2 = scr_pool.tile([P, 3, F], dt, tag="t2")
                    nc.scalar.mul(t2[:, 0, :], D[:, 2, :], -v[1])
                    nc.scalar.mul(t2[:, 1, :], D[:, 0, :], -v[2])
                    nc.scalar.mul(t2[:, 2, :], D[:, 1, :], -v[0])
                    SS = scr_pool.tile([P, 3, F], dt, tag="SS")
                    nc.vector.tensor_tensor(SS[:], t1[:], t2[:], op=Op.add)
                    dd = scr_pool.tile([P, F], dt, tag="dd")
                    nc.vector.scalar_tensor_tensor(dd[:], D[:, 1, :], v[1],
                                                   td[:], op0=Op.mult, op1=Op.add)
                    nc.vector.scalar_tensor_tensor(dd[:], D[:, 2, :], v[2],
                                                   dd[:], op0=Op.mult, op1=Op.add)
                    P1 = scr_pool.tile([P, 3, F], dt, tag="t1")
                    wb3 = w.broadcast_to((P, 3, F))
                    nc.vector.tensor_tensor(P1[:], SS[:], wb3, op=Op.mult)
                    P2 = scr_pool.tile([P, 3, F], dt, tag="SS")
                    ddb3 = dd[:].unsqueeze(1).broadcast_to((P, 3, F))
                    nc.vector.tensor_tensor(P2[:], u, ddb3, op=Op.mult)
                    nc.vector.tensor_tensor(P1[:], P1[:], P2[:], op=Op.add)
                    nc.vector.tensor_tensor(zz[:], zz[:], tprev_view, op=Op.add)
                    nc.vector.scalar_tensor_tensor(tsv, P1[:], 2.0, zz[:],
                                                   op0=Op.mult, op1=Op.add)
                nc.sync.dma_start(
                    ts_d.ap()[i].rearrange("(p f) c -> p f c", p=P), tstage[:])
                tprev_view = tsv

                # ---------------- rotations -------------------------------
                if i == 0:
                    D2 = st_pool.tile([P, 4, F], dt, tag="D")
                    for k in range(4):
                        nc.vector.tensor_scalar(D2[:, k, :], Tc(0),
                                                L['U'][k], None, op0=Op.mult)
                        nc.vector.scalar_tensor_tensor(
                            D2[:, k, :], Tsn(0), L['V'][k], D2[:, k, :],
                            op0=Op.mult, op1=Op.add)
                    D = D2
                else:
                    D = _const_rot(nc, st_pool, dt, Op, D,
                                   'z' if AXIS_IS_Z[i - 1] else 'y',
                                   L['ca'], L['sa'], pair)
                    D = _const_rot(nc, st_pool, dt, Op, D, 'x',
                                   L['cb'], L['sb'], pair)
                    D = _var_rot(nc, st_pool, scrP_pool, dt, Op, D,
                                 T_tiles[i], 'z' if AXIS_IS_Z[i] else 'y')

                # ---------------- canonicalize + stage quat ---------------
                # copy w's sign bit onto xyz: out = xyz XOR (w & 0x80000000)
                it = mybir.dt.int32
                mask = scr2_pool.tile([P, F], it, tag="flip")
                nc.vector.tensor_scalar(mask[:], D[:, 3, :].bitcast(it),
                                        -2147483648, None, op0=Op.bitwise_and)
                qstage = stage_pool.tile([P, F, 4], dt, tag="qs")
                qsv = qstage[:].transpose([0, 2, 1])
                mb3 = mask[:].unsqueeze(1).broadcast_to((P, 3, F))
                nc.vector.tensor_tensor(qsv[:, 0:3, :].bitcast(it),
                                        D[:, 0:3, :].bitcast(it), mb3,
                                        op=Op.bitwise_xor)
                nc.scalar.activation(qsv[:, 3, :], D[:, 3, :], AF.Abs)
                nc.sync.dma_start(
                    qu_d.ap()[i].rearrange("(p f) c -> p f c", p=P), qstage[:])

    nc.compile()
    return nc


def _const_rot(nc, st_pool, dt, Op, D, axis, ch, sh, pair):
    """D' = ch*D + sh*(D (x) axis_hat); 3 fused ops."""
    CD = st_pool.tile([P, 4, F], dt, tag="D")
    nc.scalar.mul(CD[:], D[:], ch)
    if axis == 'x':
        # sig_x = (w, z, -y, -x)
        nc.vector.scalar_tensor_tensor(pair(CD, 0, 1), pair(D, 3, -1), sh,
                                       pair(CD, 0, 1), op0=Op.mult, op1=Op.add)
        nc.vector.scalar_tensor_tensor(pair(CD, 2, 1), pair(D, 1, -1), -sh,
                                       pair(CD, 2, 1), op0=Op.mult, op1=Op.add)
    elif axis == 'z':
        # sig_z = (y, -x, w, -z): +s on (x,z) from (y,w); -s on (y,w) from (x,z)
        nc.vector.scalar_tensor_tensor(pair(CD, 0, 2), pair(D, 1, 2), sh,
                                       pair(CD, 0, 2), op0=Op.mult, op1=Op.add)
        nc.vector.scalar_tensor_tensor(pair(CD, 1, 2), pair(D, 0, 2), -sh,
                                       pair(CD, 1, 2), op0=Op.mult, op1=Op.add)
    else:
        # sig_y = (-z, w, x, -y): x' -= s*z; y' += s*w; z' += s*x; w' -= s*y
        nc.vector.scalar_tensor_tensor(pair(CD, 1, 1), pair(D, 3, -3), sh,
                                       pair(CD, 1, 1), op0=Op.mult, op1=Op.add)
        nc.vector.scalar_tensor_tensor(pair(CD, 0, 3), pair(D, 2, -1), -sh,
                                       pair(CD, 0, 3), op0=Op.mult, op1=Op.add)
    return CD


def _var_rot(nc, st_pool, scrP_pool, dt, Op, D, T, axis):
    """D' = c*D + s*(D (x) axis_hat); T blocks (c, s, ns, s)."""
    Tap = T[:]
    tstride = Tap.ap[0][0]
    tbase = Tap.offset
    CD = st_pool.tile([P, 4, F], dt, tag="D")
    cb4 = T[:, 0:1, :].broadcast_to((P, 4, F))
    nc.vector.tensor_tensor(CD[:], D[:], cb4, op=Op.mult)
    Pt = scrP_pool.tile([P, 4, F], dt, tag="P")
    Dap = D[:]
    dstride = Dap.ap[0][0]
    out = _ap_with(Pt[:], Pt[:].offset,
                   [[Pt[:].ap[0][0], P], [2 * F, 2], [F, 2], [1, F]])
    if axis == 'z':
        fac = _ap_with(Tap, tbase + F, [[tstride, P], [0, 2], [F, 2], [1, F]])
        op = _ap_with(Dap, Dap.offset + F,
                      [[dstride, P], [2 * F, 2], [-F, 2], [1, F]])
    else:
        fac = _ap_with(Tap, tbase + 2 * F,
                       [[tstride, P], [F, 2], [-F, 2], [1, F]])
        op = _ap_with(Dap, Dap.offset + 2 * F,
                      [[dstride, P], [-2 * F, 2], [F, 2], [1, F]])
    nc.vector.tensor_tensor(out, fac, op, op=Op.mult)
    nc.vector.tensor_tensor(CD[:], CD[:], Pt[:], op=Op.add)
    return CD


# ------------------------------------------------------------------ public
def _get_program(rot_fixed, trans_fixed):
    key = (np.asarray(rot_fixed, np.float32).tobytes(),
           np.asarray(trans_fixed, np.float32).tobytes())
    if key not in _CACHE:
        links = _precompute(rot_fixed, trans_fixed)
        nc = _build_bass(links)
        kc = np.zeros((P, 36), np.float32)
        kc[:, 0] = PI / 2
        for i in range(1, N_LINKS):
            for c in range(3):
                kc[:, 1 + 3 * (i - 1) + c] = -links[i]['tf'][c]
        for i in range(N_LINKS):
            kc[:, 19 + i] = links[i]['hbias']
        kc[:, 26] = -1.0
        for c in range(3):
            kc[:, 27 + c] = links[0]['tf'][c]
        for c in range(3):
            kc[:, 30 + c] = links[1]['K'][c]
        _CACHE[key] = (nc, kc)
    return _CACHE[key]


def run(q, rot_fixed, trans_fixed, trace=False):
    from concourse.bass_utils import run_bass_kernel_spmd
    nc, kc = _get_program(rot_fixed, trans_fixed)
    q = np.asarray(q, np.float32)
    assert q.shape == (BATCH, N_LINKS), q.shape
    in_maps = []
    for c in range(N_CORES):
        qc = np.ascontiguousarray(q[c * PER_CORE:(c + 1) * PER_CORE].T)
        in_maps.append(dict(qt=qc, kcols=kc))
    res = run_bass_kernel_spmd(nc, in_maps, core_ids=list(range(N_CORES)),
                               trace=trace)
    ts = np.empty((N_LINKS, BATCH, 3), np.float32)
    qu = np.empty((N_LINKS, BATCH, 4), np.float32)
    for c in range(N_CORES):
        ts[:, c * PER_CORE:(c + 1) * PER_CORE] = res.results[c]["ts"]
        qu[:, c * PER_CORE:(c + 1) * PER_CORE] = res.results[c]["quats"]
    return (ts, qu), res


def kernel(q, rot_fixed, trans_fixed):
    (ts, qu), _ = run(q, rot_fixed, trans_fixed, trace=False)
    return ts, qu


# revision 20
# speedup vs baseline: 1.0325x; 1.0001x over previous
"""Trainium2 Bass kernel for DifferentiableRobotModel forward kinematics.

Math: the chain quat  chain_i = qf_0 (x) Z(q_0) (x) qf_1 (x) Y(q_1) ...  is
evaluated as a sweep of single-axis rotations.  Each fixed rotation qf_i is
Euler-decomposed (host, float64) as A1(a_i) X(b_i) A3(g_i) where A3 = joint
axis of link i (z even / y odd) and A1 = joint axis of link i-1, so the
whole chain is:  per link: [A1(a_i)] [X(b_i)] [A3(g_i + q_i)] — two
constant-angle rotations + one variable-angle rotation.  A single-axis
right-multiplication on the quat state D is  D' = c*D + s*(D (x) axis_hat),
where (D (x) axis_hat) is a signed permutation of D's components —
expressible as strided access patterns, so each rotation is 3-4 fused DVE
ops on [128, 4*512] tiles.

Translations:  t_i = t_{i-1} + R(chain_{i-1}) @ tf_i  via the quat-rotate
identity  R(D)v = 2w(u x v) + 2u(u.v) + (2w^2-1)v  (unit quat), with the
constant-scale multiplies and affine terms offloaded to the Scalar engine.

Outputs match the reference's (x,y,z,w) quats canonicalized to w >= 0.

Data parallel over 8 NeuronCores: core c owns batch slice [c*65536,(c+1)*65536),
65536 = 128 partitions x 512 free elements per core.
"""
import sys
import dataclasses

sys.path.insert(0, '/opt/trn_rl_repo')

import numpy as np

N_LINKS = 7
N_CORES = 8
BATCH = 524288
PER_CORE = BATCH // N_CORES          # 65536
P = 128
F = PER_CORE // P                    # 512
PI = float(np.pi)
# joint axes: z for even links, y for odd links
AXIS_IS_Z = [True, False, True, False, True, False, True]

_CACHE = {}


# ---------------------------------------------------------------- host math
def _rx(a):
    c, s = np.cos(a), np.sin(a)
    return np.array([[1, 0, 0], [0, c, -s], [0, s, c]], dtype=np.float64)


def _ry(a):
    c, s = np.cos(a), np.sin(a)
    return np.array([[c, 0, s], [0, 1, 0], [-s, 0, c]], dtype=np.float64)


def _rz(a):
    c, s = np.cos(a), np.sin(a)
    return np.array([[c, -s, 0], [s, c, 0], [0, 0, 1]], dtype=np.float64)


def _euler_yxz(R):
    # R = Ry(a) Rx(b) Rz(g)
    b = np.arcsin(np.clip(-R[1, 2], -1, 1))
    a = np.arctan2(R[0, 2], R[2, 2])
    g = np.arctan2(R[1, 0], R[1, 1])
    return a, b, g


def _euler_zxy(R):
    # R = Rz(a) Rx(b) Ry(g)
    b = np.arcsin(np.clip(R[2, 1], -1, 1))
    a = np.arctan2(-R[0, 1], R[1, 1])
    g = np.arctan2(-R[2, 0], R[2, 2])
    return a, b, g


def _quat_from_R(R):
    # float64 rotation -> quat (x, y, z, w), any sign
    t = np.trace(R)
    if t > 0:
        w = 0.5 * np.sqrt(1 + t)
        x = (R[2, 1] - R[1, 2]) / (4 * w)
        y = (R[0, 2] - R[2, 0]) / (4 * w)
        z = (R[1, 0] - R[0, 1]) / (4 * w)
    else:
        i = int(np.argmax(np.diag(R)))
        if i == 0:
            x = 0.5 * np.sqrt(1 + R[0, 0] - R[1, 1] - R[2, 2])
            w = (R[2, 1] - R[1, 2]) / (4 * x)
            y = (R[0, 1] + R[1, 0]) / (4 * x)
            z = (R[0, 2] + R[2, 0]) / (4 * x)
        elif i == 1:
            y = 0.5 * np.sqrt(1 - R[0, 0] + R[1, 1] - R[2, 2])
            w = (R[0, 2] - R[2, 0]) / (4 * y)
            x = (R[0, 1] + R[1, 0]) / (4 * y)
            z = (R[1, 2] + R[2, 1]) / (4 * y)
        else:
            z = 0.5 * np.sqrt(1 - R[0, 0] - R[1, 1] + R[2, 2])
            w = (R[1, 0] - R[0, 1]) / (4 * z)
            x = (R[0, 2] + R[2, 0]) / (4 * z)
            y = (R[1, 2] + R[2, 1]) / (4 * z)
    return np.array([x, y, z, w])


def _reduce_half_pi(b):
    """reduce half-angle bias mod pi into [-pi/2, pi/2] (quat sign flip is
    absorbed by the output canonicalization)."""
    return float(b - PI * np.round(b / PI))


def _precompute(rot_fixed, trans_fixed):
    """Per-link constants from fp32 inputs (math in float64)."""
    rot = np.asarray(rot_fixed, dtype=np.float64)
    tf = np.asarray(trans_fixed, dtype=np.float64)
    links = []
    for i in range(N_LINKS):
        R = rot[i]
        if AXIS_IS_Z[i]:
            a, b, g = _euler_yxz(R)
            chk = _ry(a) @ _rx(b) @ _rz(g)
        else:
            a, b, g = _euler_zxy(R)
            chk = _rz(a) @ _rx(b) @ _ry(g)
        assert np.abs(chk - R).max() < 1e-5, (i, np.abs(chk - R).max())
        links.append(dict(
            alpha=a, beta=b,
            # var-rot half-angle bias, reduced into [-pi/2, pi/2]
            hbias=_reduce_half_pi(g / 2.0),
            ca=float(np.cos(a / 2)), sa=float(np.sin(a / 2)),
            cb=float(np.cos(b / 2)), sb=float(np.sin(b / 2)),
            tf=[float(v) for v in tf[i]],
        ))
    # link-1 translation via double angle: Delta_1 = cos(th)*A + sin(th)*B + C,
    # th = q_0 + 2*hbias_0;  e = Rz(-2*hbias_0) @ tf_1
    b0 = links[0]['hbias']
    e = _rz(-2.0 * b0) @ tf[1]
    Rf0 = rot[0]
    A1v = Rf0 @ np.array([e[0], e[1], 0.0])
    B1v = Rf0 @ np.array([-e[1], e[0], 0.0])
    C1v = Rf0 @ np.array([0.0, 0.0, e[2]])
    links[1]['A'] = [float(v) for v in A1v]
    links[1]['B'] = [float(v) for v in B1v]
    links[1]['K'] = [float(v) for v in (tf[0] + C1v + A1v)]
    # link 0 init quat: U = quat(Ry(a0) Rx(b0)), V = U (x) zhat
    U = _quat_from_R(_ry(links[0]['alpha']) @ _rx(links[0]['beta']))
    V = np.array([U[1], -U[0], U[3], -U[2]])
    links[0]['U'] = [float(v) for v in U]
    links[0]['V'] = [float(v) for v in V]
    return links


# ------------------------------------------------------------- bass program
def _ap_with(ap, offset, dims):
    """Build a raw AP view: same tensor, explicit [step, count] dims
    (partition dim first), offsets in elements."""
    return dataclasses.replace(ap, offset=offset, ap=type(ap.ap)(dims))


def _build_bass(links):
    import concourse.bass as bass  # noqa: F401
    from concourse import bacc
    import concourse.tile as tile
    import concourse.mybir as mybir
    from concourse.alu_op_type import AluOpType as Op

    dt = mybir.dt.float32
    AF = mybir.ActivationFunctionType

    nc = bacc.Bacc(trn_type="TRN2", target_bir_lowering=False, debug=False)

    qt_d = nc.dram_tensor("qt", [N_LINKS, PER_CORE], dt, kind="ExternalInput")
    kc_d = nc.dram_tensor("kcols", [P, 36], dt, kind="ExternalInput")
    ts_d = nc.dram_tensor("ts", [N_LINKS, PER_CORE, 3], dt, kind="ExternalOutput")
    qu_d = nc.dram_tensor("quats", [N_LINKS, PER_CORE, 4], dt, kind="ExternalOutput")

    with tile.TileContext(nc) as tc:
        with (
            tc.tile_pool(name="io", bufs=1) as io_pool,
            tc.tile_pool(name="bulk", bufs=1) as bulk_pool,
            tc.tile_pool(name="trig", bufs=3) as trig_pool,
            tc.tile_pool(name="state", bufs=5) as st_pool,
            tc.tile_pool(name="scr", bufs=1) as scr_pool,
            tc.tile_pool(name="scrP", bufs=1) as scrP_pool,
            tc.tile_pool(name="scr2", bufs=2) as scr2_pool,
            tc.tile_pool(name="stage", bufs=2) as stage_pool,
            tc.tile_pool(name="tstage", bufs=3) as tstage_pool,
        ):
            kc = io_pool.tile([P, 36], dt)
            nc.sync.dma_start(kc[:], kc_d.ap())
            PIH = kc[:, 0:1]  # pi/2 column

            qt = bulk_pool.tile([P, N_LINKS, F], dt, tag="qt")
            for i in range(N_LINKS):
                nc.sync.dma_start(
                    qt[:, i, :], qt_d.ap()[i].rearrange("(p f) -> p f", p=P))

            def block(t, k, n=1):
                return t[:, k:k + n, :]

            def pair(t, k0, step):
                base = t[:].offset
                return _ap_with(t[:], base + k0 * F,
                                [[t[:].ap[0][0], P], [step * F, 2], [1, F]])

            # ---------------- per-link trig tiles -------------------------
            # T tile per link: 4 blocks (c, s, ns, s)
            T_tiles = []
            for i in range(N_LINKS):
                h = scr2_pool.tile([P, F], dt, tag="h")
                nc.scalar.activation(h[:], qt[:, i, :], AF.Identity,
                                     bias=kc[:, 19 + i:20 + i], scale=0.5)
                g1 = scr2_pool.tile([P, F], dt, tag="g1")
                nc.vector.tensor_scalar(g1[:], h[:], 1.6, PI,
                                        op0=Op.is_gt, op1=Op.mult)
                nc.vector.tensor_tensor(h[:], h[:], g1[:], op=Op.subtract)
                g2 = scr2_pool.tile([P, F], dt, tag="g1")
                nc.vector.tensor_scalar(g2[:], h[:], -1.6, PI,
                                        op0=Op.is_lt, op1=Op.mult)
                nc.vector.tensor_tensor(h[:], h[:], g2[:], op=Op.add)
                T = trig_pool.tile([P, 4, F], dt, tag="T")
                nc.scalar.activation(T[:, 0, :], h[:], AF.Sin, bias=PIH,
                                     scale=1.0)
                nc.scalar.activation(T[:, 2, :], h[:], AF.Sin, scale=-1.0)
                hdup = h[:].unsqueeze(1).broadcast_to((P, 2, F))
                sdup = _ap_with(T[:], T[:].offset + F,
                                [[T[:].ap[0][0], P], [2 * F, 2], [1, F]])
                nc.scalar.activation(sdup, hdup, AF.Sin)
                T_tiles.append(T)

            def Tc(i):
                return T_tiles[i][:, 0, :]

            def Tsn(i):
                return T_tiles[i][:, 1, :]

            D = None
            tprev_view = None

            for i in range(N_LINKS):
                L = links[i]

                # ---------------- translation output ----------------------
                tstage = tstage_pool.tile([P, F, 3], dt, tag="ts")
                tsv = tstage[:].transpose([0, 2, 1])
                if i == 0:
                    for c in range(3):
                        nc.scalar.activation(tsv[:, c, :], qt[:, 0, :],
                                             AF.Identity,
                                             bias=kc[:, 27 + c:28 + c],
                                             scale=0.0)
                elif i == 1:
                    # Delta_1 + t_0 = K - 2A*sin(h0)^2 + 2B*sin(h0)cos(h0)
                    sq = scr2_pool.tile([P, F], dt, tag="sq")
                    nc.scalar.activation(sq[:], Tsn(0), AF.Square)
                    sc = scr2_pool.tile([P, F], dt, tag="sc")
                    nc.vector.tensor_tensor(sc[:], Tsn(0), Tc(0), op=Op.mult)
                    zl = scr_pool.tile([P, 3, F], dt, tag="zz")
                    for c in range(3):
                        nc.scalar.activation(zl[:, c, :], sq[:], AF.Identity,
                                             bias=kc[:, 30 + c:31 + c],
                                             scale=-2.0 * L['A'][c])
                        nc.vector.scalar_tensor_tensor(
                            tsv[:, c, :], sc[:], 2.0 * L['B'][c], zl[:, c, :],
                            op0=Op.mult, op1=Op.add)
                else:
                    v = L['tf']
                    u = block(D, 0, 3)
                    w = block(D, 3)
                    t1 = scr_pool.tile([P, 3, F], dt, tag="t1")
                    nc.scalar.mul(t1[:, 0, :], D[:, 1, :], v[2])
                    nc.scalar.mul(t1[:, 1, :], D[:, 2, :], v[0])
                    nc.scalar.mul(t1[:, 2, :], D[:, 0, :], v[1])
                    td = scr_pool.tile([P, F], dt, tag="td")
                    nc.scalar.mul(td[:], D[:, 0, :], v[0])
                    w2 = scr_pool.tile([P, F], dt, tag="w2")
                    nc.scalar.activation(w2[:], D[:, 3, :], AF.Square)
                    zz = scr_pool.tile([P, 3, F], dt, tag="zz")
                    for c in range(3):
                        nc.scalar.activation(zz[:, c, :], w2[:], AF.Identity,
                                             bias=kc[:, 1 + 3 * (i - 1) + c:
                                                     2 + 3 * (i - 1) + c],
                                             scale=2.0 * v[c])
                    t2 = scr_pool.tile([P, 3, F], dt, tag="t2")
                    nc.scalar.mul(t2[:, 0, :], D[:, 2, :], -v[1])
                    nc.scalar.mul(t2[:, 1, :], D[:, 0, :], -v[2])
                    nc.scalar.mul(t2[:, 2, :], D[:, 1, :], -v[0])
                    SS = scr_pool.tile([P, 3, F], dt, tag="SS")
                    nc.vector.tensor_tensor(SS[:], t1[:], t2[:], op=Op.add)
                    dd = scr_pool.tile([P, F], dt, tag="dd")
                    nc.vector.scalar_tensor_tensor(dd[:], D[:, 1, :], v[1],
                                                   td[:], op0=Op.mult, op1=Op.add)
                    nc.vector.scalar_tensor_tensor(dd[:], D[:, 2, :], v[2],
                                                   dd[:], op0=Op.mult, op1=Op.add)
                    P1 = scr_pool.tile([P, 3, F], dt, tag="t1")
                    wb3 = w.broadcast_to((P, 3, F))
                    nc.vector.tensor_tensor(P1[:], SS[:], wb3, op=Op.mult)
                    P2 = scr_pool.tile([P, 3, F], dt, tag="SS")
                    ddb3 = dd[:].unsqueeze(1).broadcast_to((P, 3, F))
                    nc.vector.tensor_tensor(P2[:], u, ddb3, op=Op.mult)
                    nc.vector.tensor_tensor(P1[:], P1[:], P2[:], op=Op.add)
                    nc.vector.tensor_tensor(zz[:], zz[:], tprev_view, op=Op.add)
                    nc.vector.scalar_tensor_tensor(tsv, P1[:], 2.0, zz[:],
                                                   op0=Op.mult, op1=Op.add)
                nc.sync.dma_start(
                    ts_d.ap()[i].rearrange("(p f) c -> p f c", p=P), tstage[:])
                tprev_view = tsv

                # ---------------- rotations -------------------------------
                if i == 0:
                    D2 = st_pool.tile([P, 4, F], dt, tag="D")
                    for k in range(4):
                        nc.vector.tensor_scalar(D2[:, k, :], Tc(0),
                                                L['U'][k], None, op0=Op.mult)
                        nc.vector.scalar_tensor_tensor(
                            D2[:, k, :], Tsn(0), L['V'][k], D2[:, k, :],
                            op0=Op.mult, op1=Op.add)
                    D = D2
                else:
                    D = _const_rot(nc, st_pool, dt, Op, D,
                                   'z' if AXIS_IS_Z[i - 1] else 'y',
                                   L['ca'], L['sa'], pair)
                    D = _const_rot(nc, st_pool, dt, Op, D, 'x',
                                   L['cb'], L['sb'], pair)
                    D = _var_rot(nc, st_pool, scrP_pool, dt, Op, D,
                                 T_tiles[i], 'z' if AXIS_IS_Z[i] else 'y')

                # ---------------- canonicalize + stage quat ---------------
                # copy w's sign bit onto xyz: out = xyz XOR (w & 0x80000000)
                it = mybir.dt.int32
                mask = scr2_pool.tile([P, F], it, tag="flip")
                nc.vector.tensor_scalar(mask[:], D[:, 3, :].bitcast(it),
                                        -2147483648, None, op0=Op.bitwise_and)
                qstage = stage_pool.tile([P, F, 4], dt, tag="qs")
                qsv = qstage[:].transpose([0, 2, 1])
                mb3 = mask[:].unsqueeze(1).broadcast_to((P, 3, F))
                nc.vector.tensor_tensor(qsv[:, 0:3, :].bitcast(it),
                                        D[:, 0:3, :].bitcast(it), mb3,
                                        op=Op.bitwise_xor)
                nc.scalar.activation(qsv[:, 3, :], D[:, 3, :], AF.Abs)
                nc.sync.dma_start(
                    qu_d.ap()[i].rearrange("(p f) c -> p f c", p=P), qstage[:])

    nc.compile()
    return nc


def _const_rot(nc, st_pool, dt, Op, D, axis, ch, sh, pair):
    """D' = ch*D + sh*(D (x) axis_hat); 3 fused ops."""
    CD = st_pool.tile([P, 4, F], dt, tag="D")
    nc.scalar.mul(CD[:], D[:], ch)
    if axis == 'x':
        # sig_x = (w, z, -y, -x)
        nc.vector.scalar_tensor_tensor(pair(CD, 0, 1), pair(D, 3, -1), sh,
                                       pair(CD, 0, 1), op0=Op.mult, op1=Op.add)
        nc.vector.scalar_tensor_tensor(pair(CD, 2, 1), pair(D, 1, -1), -sh,
                                       pair(CD, 2, 1), op0=Op.mult, op1=Op.add)
    elif axis == 'z':
        # sig_z = (y, -x, w, -z): +s on (x,z) from (y,w); -s on (y,w) from (x,z)
        nc.vector.scalar_tensor_tensor(pair(CD, 0, 2), pair(D, 1, 2), sh,
                                       pair(CD, 0, 2), op0=Op.mult, op1=Op.add)
        nc.vector.scalar_tensor_tensor(pair(CD, 1, 2), pair(D, 0, 2), -sh,
                                       pair(CD, 1, 2), op0=Op.mult, op1=Op.add)
    else:
        # sig_y = (-z, w, x, -y): x' -= s*z; y' += s*w; z' += s*x; w' -= s*y
        nc.vector.scalar_tensor_tensor(pair(CD, 1, 1), pair(D, 3, -3), sh,
                                       pair(CD, 1, 1), op0=Op.mult, op1=Op.add)
        nc.vector.scalar_tensor_tensor(pair(CD, 0, 3), pair(D, 2, -1), -sh,
                                       pair(CD, 0, 3), op0=Op.mult, op1=Op.add)
    return CD


def _var_rot(nc, st_pool, scrP_pool, dt, Op, D, T, axis):
    """D' = c*D + s*(D (x) axis_hat); T blocks (c, s, ns, s)."""
    Tap = T[:]
    tstride = Tap.ap[0][0]
    tbase = Tap.offset
    CD = st_pool.tile([P, 4, F], dt, tag="D")
    cb4 = T[:, 0:1, :].broadcast_to((P, 4, F))
    nc.vector.tensor_tensor(CD[:], D[:], cb4, op=Op.mult)
    Pt = scrP_pool.tile([P, 4, F], dt, tag="P")
    Dap = D[:]
    dstride = Dap.ap[0][0]
    out = _ap_with(Pt[:], Pt[:].offset,
                   [[Pt[:].ap[0][0], P], [2 * F, 2], [F, 2], [1, F]])
    if axis == 'z':
        fac = _ap_with(Tap, tbase + F, [[tstride, P], [0, 2], [F, 2], [1, F]])
        op = _ap_with(Dap, Dap.offset + F,
                      [[dstride, P], [2 * F, 2], [-F, 2], [1, F]])
    else:
        fac = _ap_with(Tap, tbase + 2 * F,
                       [[tstride, P], [F, 2], [-F, 2], [1, F]])
        op = _ap_with(Dap, Dap.offset + 2 * F,
                      [[dstride, P], [-2 * F, 2], [F, 2], [1, F]])
    nc.vector.tensor_tensor(out, fac, op, op=Op.mult)
    nc.vector.tensor_tensor(CD[:], CD[:], Pt[:], op=Op.add)
    return CD


# ------------------------------------------------------------------ public
def _get_program(rot_fixed, trans_fixed):
    key = (np.asarray(rot_fixed, np.float32).tobytes(),
           np.asarray(trans_fixed, np.float32).tobytes())
    if key not in _CACHE:
        links = _precompute(rot_fixed, trans_fixed)
        nc = _build_bass(links)
        kc = np.zeros((P, 36), np.float32)
        kc[:, 0] = PI / 2
        for i in range(1, N_LINKS):
            for c in range(3):
                kc[:, 1 + 3 * (i - 1) + c] = -links[i]['tf'][c]
        for i in range(N_LINKS):
            kc[:, 19 + i] = links[i]['hbias']
        kc[:, 26] = -1.0
        for c in range(3):
            kc[:, 27 + c] = links[0]['tf'][c]
        for c in range(3):
            kc[:, 30 + c] = links[1]['K'][c]
        _CACHE[key] = (nc, kc)
    return _CACHE[key]


def run(q, rot_fixed, trans_fixed, trace=False):
    from concourse.bass_utils import run_bass_kernel_spmd
    nc, kc = _get_program(rot_fixed, trans_fixed)
    q = np.asarray(q, np.float32)
    assert q.shape == (BATCH, N_LINKS), q.shape
    in_maps = []
    for c in range(N_CORES):
        qc = np.ascontiguousarray(q[c * PER_CORE:(c + 1) * PER_CORE].T)
        in_maps.append(dict(qt=qc, kcols=kc))
    res = run_bass_kernel_spmd(nc, in_maps, core_ids=list(range(N_CORES)),
                               trace=trace)
    ts = np.empty((N_LINKS, BATCH, 3), np.float32)
    qu = np.empty((N_LINKS, BATCH, 4), np.float32)
    for c in range(N_CORES):
        ts[:, c * PER_CORE:(c + 1) * PER_CORE] = res.results[c]["ts"]
        qu[:, c * PER_CORE:(c + 1) * PER_CORE] = res.results[c]["quats"]
    return (ts, qu), res


def kernel(q, rot_fixed, trans_fixed):
    (ts, qu), _ = run(q, rot_fixed, trans_fixed, trace=False)
    return ts, qu


# revision 21
# speedup vs baseline: 1.0827x; 1.0486x over previous
"""Trainium2 Bass kernel for DifferentiableRobotModel forward kinematics.

Math: the chain quat  chain_i = qf_0 (x) Z(q_0) (x) qf_1 (x) Y(q_1) ...  is
evaluated as a sweep of single-axis rotations.  Each fixed rotation qf_i is
Euler-decomposed (host, float64) as A1(a_i) X(b_i) A3(g_i) where A3 = joint
axis of link i (z even / y odd) and A1 = joint axis of link i-1, so the
whole chain is:  per link: [A1(a_i)] [X(b_i)] [A3(g_i + q_i)] — two
constant-angle rotations + one variable-angle rotation.  A single-axis
right-multiplication on the quat state D is  D' = c*D + s*(D (x) axis_hat),
where (D (x) axis_hat) is a signed permutation of D's components —
expressible as strided access patterns, so each rotation is 3-4 fused DVE
ops on [128, 4*512] tiles.

Translations:  t_i = t_{i-1} + R(chain_{i-1}) @ tf_i  via the quat-rotate
identity  R(D)v = 2w(u x v) + 2u(u.v) + (2w^2-1)v  (unit quat), with the
constant-scale multiplies and affine terms offloaded to the Scalar engine.

Outputs match the reference's (x,y,z,w) quats canonicalized to w >= 0.

Data parallel over 8 NeuronCores: core c owns batch slice [c*65536,(c+1)*65536),
65536 = 128 partitions x 512 free elements per core.
"""
import sys
import dataclasses

sys.path.insert(0, '/opt/trn_rl_repo')

import numpy as np

N_LINKS = 7
N_CORES = 8
BATCH = 524288
PER_CORE = BATCH // N_CORES          # 65536
P = 128
F = PER_CORE // P                    # 512
PI = float(np.pi)
# joint axes: z for even links, y for odd links
AXIS_IS_Z = [True, False, True, False, True, False, True]

_CACHE = {}


# ---------------------------------------------------------------- host math
def _rx(a):
    c, s = np.cos(a), np.sin(a)
    return np.array([[1, 0, 0], [0, c, -s], [0, s, c]], dtype=np.float64)


def _ry(a):
    c, s = np.cos(a), np.sin(a)
    return np.array([[c, 0, s], [0, 1, 0], [-s, 0, c]], dtype=np.float64)


def _rz(a):
    c, s = np.cos(a), np.sin(a)
    return np.array([[c, -s, 0], [s, c, 0], [0, 0, 1]], dtype=np.float64)


def _euler_yxz(R):
    # R = Ry(a) Rx(b) Rz(g)
    b = np.arcsin(np.clip(-R[1, 2], -1, 1))
    a = np.arctan2(R[0, 2], R[2, 2])
    g = np.arctan2(R[1, 0], R[1, 1])
    return a, b, g


def _euler_zxy(R):
    # R = Rz(a) Rx(b) Ry(g)
    b = np.arcsin(np.clip(R[2, 1], -1, 1))
    a = np.arctan2(-R[0, 1], R[1, 1])
    g = np.arctan2(-R[2, 0], R[2, 2])
    return a, b, g


def _quat_from_R(R):
    # float64 rotation -> quat (x, y, z, w), any sign
    t = np.trace(R)
    if t > 0:
        w = 0.5 * np.sqrt(1 + t)
        x = (R[2, 1] - R[1, 2]) / (4 * w)
        y = (R[0, 2] - R[2, 0]) / (4 * w)
        z = (R[1, 0] - R[0, 1]) / (4 * w)
    else:
        i = int(np.argmax(np.diag(R)))
        if i == 0:
            x = 0.5 * np.sqrt(1 + R[0, 0] - R[1, 1] - R[2, 2])
            w = (R[2, 1] - R[1, 2]) / (4 * x)
            y = (R[0, 1] + R[1, 0]) / (4 * x)
            z = (R[0, 2] + R[2, 0]) / (4 * x)
        elif i == 1:
            y = 0.5 * np.sqrt(1 - R[0, 0] + R[1, 1] - R[2, 2])
            w = (R[0, 2] - R[2, 0]) / (4 * y)
            x = (R[0, 1] + R[1, 0]) / (4 * y)
            z = (R[1, 2] + R[2, 1]) / (4 * y)
        else:
            z = 0.5 * np.sqrt(1 - R[0, 0] - R[1, 1] + R[2, 2])
            w = (R[1, 0] - R[0, 1]) / (4 * z)
            x = (R[0, 2] + R[2, 0]) / (4 * z)
            y = (R[1, 2] + R[2, 1]) / (4 * z)
    return np.array([x, y, z, w])


def _reduce_half_pi(b):
    """reduce half-angle bias mod pi into [-pi/2, pi/2] (quat sign flip is
    absorbed by the output canonicalization)."""
    return float(b - PI * np.round(b / PI))


def _precompute(rot_fixed, trans_fixed):
    """Per-link constants from fp32 inputs (math in float64)."""
    rot = np.asarray(rot_fixed, dtype=np.float64)
    tf = np.asarray(trans_fixed, dtype=np.float64)
    links = []
    for i in range(N_LINKS):
        R = rot[i]
        if AXIS_IS_Z[i]:
            a, b, g = _euler_yxz(R)
            chk = _ry(a) @ _rx(b) @ _rz(g)
        else:
            a, b, g = _euler_zxy(R)
            chk = _rz(a) @ _rx(b) @ _ry(g)
        assert np.abs(chk - R).max() < 1e-5, (i, np.abs(chk - R).max())
        links.append(dict(
            alpha=a, beta=b,
            # var-rot half-angle bias, reduced into [-pi/2, pi/2]
            hbias=_reduce_half_pi(g / 2.0),
            ca=float(np.cos(a / 2)), sa=float(np.sin(a / 2)),
            cb=float(np.cos(b / 2)), sb=float(np.sin(b / 2)),
            tf=[float(v) for v in tf[i]],
        ))
    # link-1 translation via double angle: Delta_1 = cos(th)*A + sin(th)*B + C,
    # th = q_0 + 2*hbias_0;  e = Rz(-2*hbias_0) @ tf_1
    b0 = links[0]['hbias']
    e = _rz(-2.0 * b0) @ tf[1]
    Rf0 = rot[0]
    A1v = Rf0 @ np.array([e[0], e[1], 0.0])
    B1v = Rf0 @ np.array([-e[1], e[0], 0.0])
    C1v = Rf0 @ np.array([0.0, 0.0, e[2]])
    links[1]['A'] = [float(v) for v in A1v]
    links[1]['B'] = [float(v) for v in B1v]
    links[1]['K'] = [float(v) for v in (tf[0] + C1v + A1v)]
    # link 0 init quat: U = quat(Ry(a0) Rx(b0)), V = U (x) zhat
    U = _quat_from_R(_ry(links[0]['alpha']) @ _rx(links[0]['beta']))
    V = np.array([U[1], -U[0], U[3], -U[2]])
    links[0]['U'] = [float(v) for v in U]
    links[0]['V'] = [float(v) for v in V]
    return links


# ------------------------------------------------------------- bass program
def _ap_with(ap, offset, dims):
    """Build a raw AP view: same tensor, explicit [step, count] dims
    (partition dim first), offsets in elements."""
    return dataclasses.replace(ap, offset=offset, ap=type(ap.ap)(dims))


def _build_bass(links):
    import concourse.bass as bass  # noqa: F401
    from concourse import bacc
    import concourse.tile as tile
    import concourse.mybir as mybir
    from concourse.alu_op_type import AluOpType as Op

    dt = mybir.dt.float32
    AF = mybir.ActivationFunctionType

    nc = bacc.Bacc(trn_type="TRN2", target_bir_lowering=False, debug=False)

    qt_d = nc.dram_tensor("qt", [N_LINKS, PER_CORE], dt, kind="ExternalInput")
    kc_d = nc.dram_tensor("kcols", [P, 36], dt, kind="ExternalInput")
    ts_d = nc.dram_tensor("ts", [N_LINKS, PER_CORE, 3], dt, kind="ExternalOutput")
    qu_d = nc.dram_tensor("quats", [N_LINKS, PER_CORE, 4], dt, kind="ExternalOutput")

    with tile.TileContext(nc) as tc:
        with (
            tc.tile_pool(name="io", bufs=1) as io_pool,
            tc.tile_pool(name="bulk", bufs=1) as bulk_pool,
            tc.tile_pool(name="trig", bufs=3) as trig_pool,
            tc.tile_pool(name="state", bufs=5) as st_pool,
            tc.tile_pool(name="scr", bufs=1) as scr_pool,
            tc.tile_pool(name="scrP", bufs=1) as scrP_pool,
            tc.tile_pool(name="scr2", bufs=2) as scr2_pool,
            tc.tile_pool(name="stage", bufs=2) as stage_pool,
            tc.tile_pool(name="tstage", bufs=3) as tstage_pool,
        ):
            kc = io_pool.tile([P, 36], dt)
            nc.sync.dma_start(kc[:], kc_d.ap())
            PIH = kc[:, 0:1]  # pi/2 column

            qt = bulk_pool.tile([P, N_LINKS, F], dt, tag="qt")
            for i in range(N_LINKS):
                nc.sync.dma_start(
                    qt[:, i, :], qt_d.ap()[i].rearrange("(p f) -> p f", p=P))

            def block(t, k, n=1):
                return t[:, k:k + n, :]

            def pair(t, k0, step):
                base = t[:].offset
                return _ap_with(t[:], base + k0 * F,
                                [[t[:].ap[0][0], P], [step * F, 2], [1, F]])

            # ---------------- per-link trig tiles -------------------------
            # T tile per link: 4 blocks (c, s, ns, s)
            T_tiles = []
            for i in range(N_LINKS):
                h = scr2_pool.tile([P, F], dt, tag="h")
                nc.scalar.activation(h[:], qt[:, i, :], AF.Identity,
                                     bias=kc[:, 19 + i:20 + i], scale=0.5)
                g1 = scr2_pool.tile([P, F], dt, tag="g1")
                nc.vector.tensor_scalar(g1[:], h[:], 1.6, PI,
                                        op0=Op.is_gt, op1=Op.mult)
                nc.vector.tensor_tensor(h[:], h[:], g1[:], op=Op.subtract)
                g2 = scr2_pool.tile([P, F], dt, tag="g1")
                nc.vector.tensor_scalar(g2[:], h[:], -1.6, PI,
                                        op0=Op.is_lt, op1=Op.mult)
                nc.vector.tensor_tensor(h[:], h[:], g2[:], op=Op.add)
                T = trig_pool.tile([P, 4, F], dt, tag="T")
                nc.scalar.activation(T[:, 0, :], h[:], AF.Sin, bias=PIH,
                                     scale=1.0)
                nc.scalar.activation(T[:, 2, :], h[:], AF.Sin, scale=-1.0)
                hdup = h[:].unsqueeze(1).broadcast_to((P, 2, F))
                sdup = _ap_with(T[:], T[:].offset + F,
                                [[T[:].ap[0][0], P], [2 * F, 2], [1, F]])
                nc.scalar.activation(sdup, hdup, AF.Sin)
                T_tiles.append(T)

            def Tc(i):
                return T_tiles[i][:, 0, :]

            def Tsn(i):
                return T_tiles[i][:, 1, :]

            D = None
            tprev_view = None

            for i in range(N_LINKS):
                L = links[i]

                Dprev = D
                # ---------------- rotations -------------------------------
                if i == 0:
                    D2 = st_pool.tile([P, 4, F], dt, tag="D")
                    for k in range(4):
                        nc.vector.tensor_scalar(D2[:, k, :], Tc(0),
                                                L['U'][k], None, op0=Op.mult)
                        nc.vector.scalar_tensor_tensor(
                            D2[:, k, :], Tsn(0), L['V'][k], D2[:, k, :],
                            op0=Op.mult, op1=Op.add)
                    D = D2
                else:
                    D = _const_rot(nc, st_pool, dt, Op, D,
                                   'z' if AXIS_IS_Z[i - 1] else 'y',
                                   L['ca'], L['sa'], pair)
                    D = _const_rot(nc, st_pool, dt, Op, D, 'x',
                                   L['cb'], L['sb'], pair)
                    D = _var_rot(nc, st_pool, scrP_pool, dt, Op, D,
                                 T_tiles[i], 'z' if AXIS_IS_Z[i] else 'y')

                # ---------------- canonicalize + stage quat ---------------
                # copy w's sign bit onto xyz: out = xyz XOR (w & 0x80000000)
                it = mybir.dt.int32
                mask = scr2_pool.tile([P, F], it, tag="flip")
                nc.vector.tensor_scalar(mask[:], D[:, 3, :].bitcast(it),
                                        -2147483648, None, op0=Op.bitwise_and)
                qstage = stage_pool.tile([P, F, 4], dt, tag="qs")
                qsv = qstage[:].transpose([0, 2, 1])
                mb3 = mask[:].unsqueeze(1).broadcast_to((P, 3, F))
                nc.vector.tensor_tensor(qsv[:, 0:3, :].bitcast(it),
                                        D[:, 0:3, :].bitcast(it), mb3,
                                        op=Op.bitwise_xor)
                nc.scalar.activation(qsv[:, 3, :], D[:, 3, :], AF.Abs)
                nc.sync.dma_start(
                    qu_d.ap()[i].rearrange("(p f) c -> p f c", p=P), qstage[:])

                # ---------------- translation output ----------------------
                tstage = tstage_pool.tile([P, F, 3], dt, tag="ts")
                tsv = tstage[:].transpose([0, 2, 1])
                if i == 0:
                    for c in range(3):
                        nc.scalar.activation(tsv[:, c, :], qt[:, 0, :],
                                             AF.Identity,
                                             bias=kc[:, 27 + c:28 + c],
                                             scale=0.0)
                elif i == 1:
                    # Delta_1 + t_0 = K - 2A*sin(h0)^2 + 2B*sin(h0)cos(h0)
                    sq = scr2_pool.tile([P, F], dt, tag="sq")
                    nc.scalar.activation(sq[:], Tsn(0), AF.Square)
                    sc = scr2_pool.tile([P, F], dt, tag="sc")
                    nc.vector.tensor_tensor(sc[:], Tsn(0), Tc(0), op=Op.mult)
                    zl = scr_pool.tile([P, 3, F], dt, tag="zz")
                    for c in range(3):
                        nc.scalar.activation(zl[:, c, :], sq[:], AF.Identity,
                                             bias=kc[:, 30 + c:31 + c],
                                             scale=-2.0 * L['A'][c])
                        nc.vector.scalar_tensor_tensor(
                            tsv[:, c, :], sc[:], 2.0 * L['B'][c], zl[:, c, :],
                            op0=Op.mult, op1=Op.add)
                else:
                    v = L['tf']
                    u = block(Dprev, 0, 3)
                    w = block(Dprev, 3)
                    t1 = scr_pool.tile([P, 3, F], dt, tag="t1")
                    nc.scalar.mul(t1[:, 0, :], Dprev[:, 1, :], v[2])
                    nc.scalar.mul(t1[:, 1, :], Dprev[:, 2, :], v[0])
                    nc.scalar.mul(t1[:, 2, :], Dprev[:, 0, :], v[1])
                    td = scr_pool.tile([P, F], dt, tag="td")
                    nc.scalar.mul(td[:], Dprev[:, 0, :], v[0])
                    w2 = scr_pool.tile([P, F], dt, tag="w2")
                    nc.scalar.activation(w2[:], Dprev[:, 3, :], AF.Square)
                    zz = scr_pool.tile([P, 3, F], dt, tag="zz")
                    for c in range(3):
                        nc.scalar.activation(zz[:, c, :], w2[:], AF.Identity,
                                             bias=kc[:, 1 + 3 * (i - 1) + c:
                                                     2 + 3 * (i - 1) + c],
                                             scale=2.0 * v[c])
                    t2 = scr_pool.tile([P, 3, F], dt, tag="t2")
                    nc.scalar.mul(t2[:, 0, :], Dprev[:, 2, :], -v[1])
                    nc.scalar.mul(t2[:, 1, :], Dprev[:, 0, :], -v[2])
                    nc.scalar.mul(t2[:, 2, :], Dprev[:, 1, :], -v[0])
                    SS = scr_pool.tile([P, 3, F], dt, tag="SS")
                    nc.vector.tensor_tensor(SS[:], t1[:], t2[:], op=Op.add)
                    dd = scr_pool.tile([P, F], dt, tag="dd")
                    nc.vector.scalar_tensor_tensor(dd[:], Dprev[:, 1, :], v[1],
                                                   td[:], op0=Op.mult, op1=Op.add)
                    nc.vector.scalar_tensor_tensor(dd[:], Dprev[:, 2, :], v[2],
                                                   dd[:], op0=Op.mult, op1=Op.add)
                    P1 = scr_pool.tile([P, 3, F], dt, tag="t1")
                    wb3 = w.broadcast_to((P, 3, F))
                    nc.vector.tensor_tensor(P1[:], SS[:], wb3, op=Op.mult)
                    P2 = scr_pool.tile([P, 3, F], dt, tag="SS")
                    ddb3 = dd[:].unsqueeze(1).broadcast_to((P, 3, F))
                    nc.vector.tensor_tensor(P2[:], u, ddb3, op=Op.mult)
                    nc.vector.tensor_tensor(P1[:], P1[:], P2[:], op=Op.add)
                    nc.vector.tensor_tensor(zz[:], zz[:], tprev_view, op=Op.add)
                    nc.vector.scalar_tensor_tensor(tsv, P1[:], 2.0, zz[:],
                                                   op0=Op.mult, op1=Op.add)
                nc.sync.dma_start(
                    ts_d.ap()[i].rearrange("(p f) c -> p f c", p=P), tstage[:])
                tprev_view = tsv

    nc.compile()
    return nc


def _const_rot(nc, st_pool, dt, Op, D, axis, ch, sh, pair):
    """D' = ch*D + sh*(D (x) axis_hat); 3 fused ops."""
    CD = st_pool.tile([P, 4, F], dt, tag="D")
    nc.scalar.mul(CD[:], D[:], ch)
    if axis == 'x':
        # sig_x = (w, z, -y, -x)
        nc.vector.scalar_tensor_tensor(pair(CD, 0, 1), pair(D, 3, -1), sh,
                                       pair(CD, 0, 1), op0=Op.mult, op1=Op.add)
        nc.vector.scalar_tensor_tensor(pair(CD, 2, 1), pair(D, 1, -1), -sh,
                                       pair(CD, 2, 1), op0=Op.mult, op1=Op.add)
    elif axis == 'z':
        # sig_z = (y, -x, w, -z): +s on (x,z) from (y,w); -s on (y,w) from (x,z)
        nc.vector.scalar_tensor_tensor(pair(CD, 0, 2), pair(D, 1, 2), sh,
                                       pair(CD, 0, 2), op0=Op.mult, op1=Op.add)
        nc.vector.scalar_tensor_tensor(pair(CD, 1, 2), pair(D, 0, 2), -sh,
                                       pair(CD, 1, 2), op0=Op.mult, op1=Op.add)
    else:
        # sig_y = (-z, w, x, -y): x' -= s*z; y' += s*w; z' += s*x; w' -= s*y
        nc.vector.scalar_tensor_tensor(pair(CD, 1, 1), pair(D, 3, -3), sh,
                                       pair(CD, 1, 1), op0=Op.mult, op1=Op.add)
        nc.vector.scalar_tensor_tensor(pair(CD, 0, 3), pair(D, 2, -1), -sh,
                                       pair(CD, 0, 3), op0=Op.mult, op1=Op.add)
    return CD


def _var_rot(nc, st_pool, scrP_pool, dt, Op, D, T, axis):
    """D' = c*D + s*(D (x) axis_hat); T blocks (c, s, ns, s)."""
    Tap = T[:]
    tstride = Tap.ap[0][0]
    tbase = Tap.offset
    CD = st_pool.tile([P, 4, F], dt, tag="D")
    cb4 = T[:, 0:1, :].broadcast_to((P, 4, F))
    nc.vector.tensor_tensor(CD[:], D[:], cb4, op=Op.mult)
    Pt = scrP_pool.tile([P, 4, F], dt, tag="P")
    Dap = D[:]
    dstride = Dap.ap[0][0]
    out = _ap_with(Pt[:], Pt[:].offset,
                   [[Pt[:].ap[0][0], P], [2 * F, 2], [F, 2], [1, F]])
    if axis == 'z':
        fac = _ap_with(Tap, tbase + F, [[tstride, P], [0, 2], [F, 2], [1, F]])
        op = _ap_with(Dap, Dap.offset + F,
                      [[dstride, P], [2 * F, 2], [-F, 2], [1, F]])
    else:
        fac = _ap_with(Tap, tbase + 2 * F,
                       [[tstride, P], [F, 2], [-F, 2], [1, F]])
        op = _ap_with(Dap, Dap.offset + 2 * F,
                      [[dstride, P], [-2 * F, 2], [F, 2], [1, F]])
    nc.vector.tensor_tensor(out, fac, op, op=Op.mult)
    nc.vector.tensor_tensor(CD[:], CD[:], Pt[:], op=Op.add)
    return CD


# ------------------------------------------------------------------ public
def _get_program(rot_fixed, trans_fixed):
    key = (np.asarray(rot_fixed, np.float32).tobytes(),
           np.asarray(trans_fixed, np.float32).tobytes())
    if key not in _CACHE:
        links = _precompute(rot_fixed, trans_fixed)
        nc = _build_bass(links)
        kc = np.zeros((P, 36), np.float32)
        kc[:, 0] = PI / 2
        for i in range(1, N_LINKS):
            for c in range(3):
                kc[:, 1 + 3 * (i - 1) + c] = -links[i]['tf'][c]
        for i in range(N_LINKS):
            kc[:, 19 + i] = links[i]['hbias']
        kc[:, 26] = -1.0
        for c in range(3):
            kc[:, 27 + c] = links[0]['tf'][c]
        for c in range(3):
            kc[:, 30 + c] = links[1]['K'][c]
        _CACHE[key] = (nc, kc)
    return _CACHE[key]


def run(q, rot_fixed, trans_fixed, trace=False):
    from concourse.bass_utils import run_bass_kernel_spmd
    nc, kc = _get_program(rot_fixed, trans_fixed)
    q = np.asarray(q, np.float32)
    assert q.shape == (BATCH, N_LINKS), q.shape
    in_maps = []
    for c in range(N_CORES):
        qc = np.ascontiguousarray(q[c * PER_CORE:(c + 1) * PER_CORE].T)
        in_maps.append(dict(qt=qc, kcols=kc))
    res = run_bass_kernel_spmd(nc, in_maps, core_ids=list(range(N_CORES)),
                               trace=trace)
    ts = np.empty((N_LINKS, BATCH, 3), np.float32)
    qu = np.empty((N_LINKS, BATCH, 4), np.float32)
    for c in range(N_CORES):
        ts[:, c * PER_CORE:(c + 1) * PER_CORE] = res.results[c]["ts"]
        qu[:, c * PER_CORE:(c + 1) * PER_CORE] = res.results[c]["quats"]
    return (ts, qu), res


def kernel(q, rot_fixed, trans_fixed):
    (ts, qu), _ = run(q, rot_fixed, trans_fixed, trace=False)
    return ts, qu


# revision 22
# speedup vs baseline: 1.1012x; 1.0171x over previous
"""Trainium2 Bass kernel for DifferentiableRobotModel forward kinematics.

Math: the chain quat  chain_i = qf_0 (x) Z(q_0) (x) qf_1 (x) Y(q_1) ...  is
evaluated as a sweep of single-axis rotations.  Each fixed rotation qf_i is
Euler-decomposed (host, float64) as A1(a_i) X(b_i) A3(g_i) where A3 = joint
axis of link i (z even / y odd) and A1 = joint axis of link i-1, so the
whole chain is:  per link: [A1(a_i)] [X(b_i)] [A3(g_i + q_i)] — two
constant-angle rotations + one variable-angle rotation.  A single-axis
right-multiplication on the quat state D is  D' = c*D + s*(D (x) axis_hat),
where (D (x) axis_hat) is a signed permutation of D's components —
expressible as strided access patterns, so each rotation is 3-4 fused DVE
ops on [128, 4*512] tiles.

Translations:  t_i = t_{i-1} + R(chain_{i-1}) @ tf_i  via the quat-rotate
identity  R(D)v = 2w(u x v) + 2u(u.v) + (2w^2-1)v  (unit quat), with the
constant-scale multiplies and affine terms offloaded to the Scalar engine.

Outputs match the reference's (x,y,z,w) quats canonicalized to w >= 0.

Data parallel over 8 NeuronCores: core c owns batch slice [c*65536,(c+1)*65536),
65536 = 128 partitions x 512 free elements per core.
"""
import sys
import dataclasses

sys.path.insert(0, '/opt/trn_rl_repo')

import numpy as np

N_LINKS = 7
N_CORES = 8
BATCH = 524288
PER_CORE = BATCH // N_CORES          # 65536
P = 128
F = PER_CORE // P                    # 512
PI = float(np.pi)
# joint axes: z for even links, y for odd links
AXIS_IS_Z = [True, False, True, False, True, False, True]

_CACHE = {}


# ---------------------------------------------------------------- host math
def _rx(a):
    c, s = np.cos(a), np.sin(a)
    return np.array([[1, 0, 0], [0, c, -s], [0, s, c]], dtype=np.float64)


def _ry(a):
    c, s = np.cos(a), np.sin(a)
    return np.array([[c, 0, s], [0, 1, 0], [-s, 0, c]], dtype=np.float64)


def _rz(a):
    c, s = np.cos(a), np.sin(a)
    return np.array([[c, -s, 0], [s, c, 0], [0, 0, 1]], dtype=np.float64)


def _euler_yxz(R):
    # R = Ry(a) Rx(b) Rz(g)
    b = np.arcsin(np.clip(-R[1, 2], -1, 1))
    a = np.arctan2(R[0, 2], R[2, 2])
    g = np.arctan2(R[1, 0], R[1, 1])
    return a, b, g


def _euler_zxy(R):
    # R = Rz(a) Rx(b) Ry(g)
    b = np.arcsin(np.clip(R[2, 1], -1, 1))
    a = np.arctan2(-R[0, 1], R[1, 1])
    g = np.arctan2(-R[2, 0], R[2, 2])
    return a, b, g


def _quat_from_R(R):
    # float64 rotation -> quat (x, y, z, w), any sign
    t = np.trace(R)
    if t > 0:
        w = 0.5 * np.sqrt(1 + t)
        x = (R[2, 1] - R[1, 2]) / (4 * w)
        y = (R[0, 2] - R[2, 0]) / (4 * w)
        z = (R[1, 0] - R[0, 1]) / (4 * w)
    else:
        i = int(np.argmax(np.diag(R)))
        if i == 0:
            x = 0.5 * np.sqrt(1 + R[0, 0] - R[1, 1] - R[2, 2])
            w = (R[2, 1] - R[1, 2]) / (4 * x)
            y = (R[0, 1] + R[1, 0]) / (4 * x)
            z = (R[0, 2] + R[2, 0]) / (4 * x)
        elif i == 1:
            y = 0.5 * np.sqrt(1 - R[0, 0] + R[1, 1] - R[2, 2])
            w = (R[0, 2] - R[2, 0]) / (4 * y)
            x = (R[0, 1] + R[1, 0]) / (4 * y)
            z = (R[1, 2] + R[2, 1]) / (4 * y)
        else:
            z = 0.5 * np.sqrt(1 - R[0, 0] - R[1, 1] + R[2, 2])
            w = (R[1, 0] - R[0, 1]) / (4 * z)
            x = (R[0, 2] + R[2, 0]) / (4 * z)
            y = (R[1, 2] + R[2, 1]) / (4 * z)
    return np.array([x, y, z, w])


def _reduce_half_pi(b):
    """reduce half-angle bias mod pi into [-pi/2, pi/2] (quat sign flip is
    absorbed by the output canonicalization)."""
    return float(b - PI * np.round(b / PI))


def _precompute(rot_fixed, trans_fixed):
    """Per-link constants from fp32 inputs (math in float64)."""
    rot = np.asarray(rot_fixed, dtype=np.float64)
    tf = np.asarray(trans_fixed, dtype=np.float64)
    links = []
    for i in range(N_LINKS):
        R = rot[i]
        if AXIS_IS_Z[i]:
            a, b, g = _euler_yxz(R)
            chk = _ry(a) @ _rx(b) @ _rz(g)
        else:
            a, b, g = _euler_zxy(R)
            chk = _rz(a) @ _rx(b) @ _ry(g)
        assert np.abs(chk - R).max() < 1e-5, (i, np.abs(chk - R).max())
        links.append(dict(
            alpha=a, beta=b,
            # var-rot half-angle bias, reduced into [-pi/2, pi/2]
            hbias=_reduce_half_pi(g / 2.0),
            ca=float(np.cos(a / 2)), sa=float(np.sin(a / 2)),
            cb=float(np.cos(b / 2)), sb=float(np.sin(b / 2)),
            tf=[float(v) for v in tf[i]],
        ))
    # link-1 translation via double angle: Delta_1 = cos(th)*A + sin(th)*B + C,
    # th = q_0 + 2*hbias_0;  e = Rz(-2*hbias_0) @ tf_1
    b0 = links[0]['hbias']
    e = _rz(-2.0 * b0) @ tf[1]
    Rf0 = rot[0]
    A1v = Rf0 @ np.array([e[0], e[1], 0.0])
    B1v = Rf0 @ np.array([-e[1], e[0], 0.0])
    C1v = Rf0 @ np.array([0.0, 0.0, e[2]])
    links[1]['A'] = [float(v) for v in A1v]
    links[1]['B'] = [float(v) for v in B1v]
    links[1]['K'] = [float(v) for v in (tf[0] + C1v + A1v)]
    # link 0 init quat: U = quat(Ry(a0) Rx(b0)), V = U (x) zhat
    U = _quat_from_R(_ry(links[0]['alpha']) @ _rx(links[0]['beta']))
    V = np.array([U[1], -U[0], U[3], -U[2]])
    links[0]['U'] = [float(v) for v in U]
    links[0]['V'] = [float(v) for v in V]
    return links


# ------------------------------------------------------------- bass program
def _ap_with(ap, offset, dims):
    """Build a raw AP view: same tensor, explicit [step, count] dims
    (partition dim first), offsets in elements."""
    return dataclasses.replace(ap, offset=offset, ap=type(ap.ap)(dims))


def _build_bass(links):
    import concourse.bass as bass  # noqa: F401
    from concourse import bacc
    import concourse.tile as tile
    import concourse.mybir as mybir
    from concourse.alu_op_type import AluOpType as Op

    dt = mybir.dt.float32
    AF = mybir.ActivationFunctionType

    nc = bacc.Bacc(trn_type="TRN2", target_bir_lowering=False, debug=False)

    qt_d = nc.dram_tensor("qt", [N_LINKS, PER_CORE], dt, kind="ExternalInput")
    kc_d = nc.dram_tensor("kcols", [P, 36], dt, kind="ExternalInput")
    ts_d = nc.dram_tensor("ts", [N_LINKS, PER_CORE, 3], dt, kind="ExternalOutput")
    qu_d = nc.dram_tensor("quats", [N_LINKS, PER_CORE, 4], dt, kind="ExternalOutput")

    with tile.TileContext(nc) as tc:
        with (
            tc.tile_pool(name="io", bufs=1) as io_pool,
            tc.tile_pool(name="bulk", bufs=1) as bulk_pool,
            tc.tile_pool(name="trig", bufs=3) as trig_pool,
            tc.tile_pool(name="state", bufs=5) as st_pool,
            tc.tile_pool(name="scr", bufs=1) as scr_pool,
            tc.tile_pool(name="scrP", bufs=1) as scrP_pool,
            tc.tile_pool(name="scr2", bufs=2) as scr2_pool,
            tc.tile_pool(name="stage", bufs=2) as stage_pool,
            tc.tile_pool(name="tstage", bufs=3) as tstage_pool,
        ):
            kc = io_pool.tile([P, 36], dt)
            nc.sync.dma_start(kc[:], kc_d.ap())
            PIH = kc[:, 0:1]  # pi/2 column

            qt = bulk_pool.tile([P, N_LINKS, F], dt, tag="qt")
            for i in range(N_LINKS):
                nc.sync.dma_start(
                    qt[:, i, :], qt_d.ap()[i].rearrange("(p f) -> p f", p=P))

            def block(t, k, n=1):
                return t[:, k:k + n, :]

            def pair(t, k0, step):
                base = t[:].offset
                return _ap_with(t[:], base + k0 * F,
                                [[t[:].ap[0][0], P], [step * F, 2], [1, F]])

            # ---------------- per-link trig tiles -------------------------
            # T tile per link: 4 blocks (c, s, ns, s)
            T_tiles = []
            for i in range(N_LINKS):
                h = scr2_pool.tile([P, F], dt, tag="h")
                nc.scalar.activation(h[:], qt[:, i, :], AF.Identity,
                                     bias=kc[:, 19 + i:20 + i], scale=0.5)
                g1 = scr2_pool.tile([P, F], dt, tag="g1")
                nc.vector.tensor_scalar(g1[:], h[:], 1.6, PI,
                                        op0=Op.is_gt, op1=Op.mult)
                nc.vector.tensor_tensor(h[:], h[:], g1[:], op=Op.subtract)
                g2 = scr2_pool.tile([P, F], dt, tag="g1")
                nc.vector.tensor_scalar(g2[:], h[:], -1.6, PI,
                                        op0=Op.is_lt, op1=Op.mult)
                nc.vector.tensor_tensor(h[:], h[:], g2[:], op=Op.add)
                T = trig_pool.tile([P, 4, F], dt, tag="T")
                nc.scalar.activation(T[:, 0, :], h[:], AF.Sin, bias=PIH,
                                     scale=1.0)
                nc.scalar.activation(T[:, 2, :], h[:], AF.Sin, scale=-1.0)
                hdup = h[:].unsqueeze(1).broadcast_to((P, 2, F))
                sdup = _ap_with(T[:], T[:].offset + F,
                                [[T[:].ap[0][0], P], [2 * F, 2], [1, F]])
                nc.scalar.activation(sdup, hdup, AF.Sin)
                T_tiles.append(T)

            def Tc(i):
                return T_tiles[i][:, 0, :]

            def Tsn(i):
                return T_tiles[i][:, 1, :]

            D = None
            tprev_view = None

            for i in range(N_LINKS):
                L = links[i]

                Dprev = D
                if i >= 2:
                    v = links[i]['tf']
                    t1 = scr_pool.tile([P, 3, F], dt, tag="t1")
                    nc.scalar.mul(t1[:, 0, :], Dprev[:, 1, :], v[2])
                    nc.scalar.mul(t1[:, 1, :], Dprev[:, 2, :], v[0])
                    nc.scalar.mul(t1[:, 2, :], Dprev[:, 0, :], v[1])
                    t2 = scr_pool.tile([P, 3, F], dt, tag="t2")
                    nc.scalar.mul(t2[:, 0, :], Dprev[:, 2, :], -v[1])
                    nc.scalar.mul(t2[:, 1, :], Dprev[:, 0, :], -v[2])
                    nc.scalar.mul(t2[:, 2, :], Dprev[:, 1, :], -v[0])
                    td = scr_pool.tile([P, F], dt, tag="td")
                    nc.scalar.mul(td[:], Dprev[:, 0, :], v[0])
                    w2 = scr_pool.tile([P, F], dt, tag="w2")
                    nc.scalar.activation(w2[:], Dprev[:, 3, :], AF.Square)
                    zz = scr_pool.tile([P, 3, F], dt, tag="zz")
                    for c in range(3):
                        nc.scalar.activation(zz[:, c, :], w2[:], AF.Identity,
                                             bias=kc[:, 1 + 3 * (i - 1) + c:
                                                     2 + 3 * (i - 1) + c],
                                             scale=2.0 * v[c])
                # ---------------- rotations -------------------------------
                if i == 0:
                    D2 = st_pool.tile([P, 4, F], dt, tag="D")
                    for k in range(4):
                        nc.vector.tensor_scalar(D2[:, k, :], Tc(0),
                                                L['U'][k], None, op0=Op.mult)
                        nc.vector.scalar_tensor_tensor(
                            D2[:, k, :], Tsn(0), L['V'][k], D2[:, k, :],
                            op0=Op.mult, op1=Op.add)
                    D = D2
                else:
                    D = _const_rot(nc, st_pool, dt, Op, D,
                                   'z' if AXIS_IS_Z[i - 1] else 'y',
                                   L['ca'], L['sa'], pair)
                    D = _const_rot(nc, st_pool, dt, Op, D, 'x',
                                   L['cb'], L['sb'], pair)
                    D = _var_rot(nc, st_pool, scrP_pool, dt, Op, D,
                                 T_tiles[i], 'z' if AXIS_IS_Z[i] else 'y')

                # ---------------- canonicalize + stage quat ---------------
                # copy w's sign bit onto xyz: out = xyz XOR (w & 0x80000000)
                it = mybir.dt.int32
                mask = scr2_pool.tile([P, F], it, tag="flip")
                nc.vector.tensor_scalar(mask[:], D[:, 3, :].bitcast(it),
                                        -2147483648, None, op0=Op.bitwise_and)
                qstage = stage_pool.tile([P, F, 4], dt, tag="qs")
                qsv = qstage[:].transpose([0, 2, 1])
                mb3 = mask[:].unsqueeze(1).broadcast_to((P, 3, F))
                nc.vector.tensor_tensor(qsv[:, 0:3, :].bitcast(it),
                                        D[:, 0:3, :].bitcast(it), mb3,
                                        op=Op.bitwise_xor)
                nc.scalar.activation(qsv[:, 3, :], D[:, 3, :], AF.Abs)
                nc.sync.dma_start(
                    qu_d.ap()[i].rearrange("(p f) c -> p f c", p=P), qstage[:])

                # ---------------- translation output ----------------------
                tstage = tstage_pool.tile([P, F, 3], dt, tag="ts")
                tsv = tstage[:].transpose([0, 2, 1])
                if i == 0:
                    for c in range(3):
                        nc.scalar.activation(tsv[:, c, :], qt[:, 0, :],
                                             AF.Identity,
                                             bias=kc[:, 27 + c:28 + c],
                                             scale=0.0)
                elif i == 1:
                    # Delta_1 + t_0 = K - 2A*sin(h0)^2 + 2B*sin(h0)cos(h0)
                    sq = scr2_pool.tile([P, F], dt, tag="sq")
                    nc.scalar.activation(sq[:], Tsn(0), AF.Square)
                    sc = scr2_pool.tile([P, F], dt, tag="sc")
                    nc.vector.tensor_tensor(sc[:], Tsn(0), Tc(0), op=Op.mult)
                    zl = scr_pool.tile([P, 3, F], dt, tag="zz")
                    for c in range(3):
                        nc.scalar.activation(zl[:, c, :], sq[:], AF.Identity,
                                             bias=kc[:, 30 + c:31 + c],
                                             scale=-2.0 * L['A'][c])
                        nc.vector.scalar_tensor_tensor(
                            tsv[:, c, :], sc[:], 2.0 * L['B'][c], zl[:, c, :],
                            op0=Op.mult, op1=Op.add)
                else:
                    u = block(Dprev, 0, 3)
                    w = block(Dprev, 3)
                    SS = scr_pool.tile([P, 3, F], dt, tag="SS")
                    nc.vector.tensor_tensor(SS[:], t1[:], t2[:], op=Op.add)
                    dd = scr_pool.tile([P, F], dt, tag="dd")
                    nc.vector.scalar_tensor_tensor(dd[:], Dprev[:, 1, :], v[1],
                                                   td[:], op0=Op.mult, op1=Op.add)
                    nc.vector.scalar_tensor_tensor(dd[:], Dprev[:, 2, :], v[2],
                                                   dd[:], op0=Op.mult, op1=Op.add)
                    P1 = scr_pool.tile([P, 3, F], dt, tag="t1")
                    wb3 = w.broadcast_to((P, 3, F))
                    nc.vector.tensor_tensor(P1[:], SS[:], wb3, op=Op.mult)
                    P2 = scr_pool.tile([P, 3, F], dt, tag="SS")
                    ddb3 = dd[:].unsqueeze(1).broadcast_to((P, 3, F))
                    nc.vector.tensor_tensor(P2[:], u, ddb3, op=Op.mult)
                    nc.vector.tensor_tensor(P1[:], P1[:], P2[:], op=Op.add)
                    nc.vector.tensor_tensor(zz[:], zz[:], tprev_view, op=Op.add)
                    nc.vector.scalar_tensor_tensor(tsv, P1[:], 2.0, zz[:],
                                                   op0=Op.mult, op1=Op.add)
                nc.sync.dma_start(
                    ts_d.ap()[i].rearrange("(p f) c -> p f c", p=P), tstage[:])
                tprev_view = tsv

    nc.compile()
    return nc


def _const_rot(nc, st_pool, dt, Op, D, axis, ch, sh, pair):
    """D' = ch*D + sh*(D (x) axis_hat); 3 fused ops."""
    CD = st_pool.tile([P, 4, F], dt, tag="D")
    nc.scalar.mul(CD[:], D[:], ch)
    if axis == 'x':
        # sig_x = (w, z, -y, -x)
        nc.vector.scalar_tensor_tensor(pair(CD, 0, 1), pair(D, 3, -1), sh,
                                       pair(CD, 0, 1), op0=Op.mult, op1=Op.add)
        nc.vector.scalar_tensor_tensor(pair(CD, 2, 1), pair(D, 1, -1), -sh,
                                       pair(CD, 2, 1), op0=Op.mult, op1=Op.add)
    elif axis == 'z':
        # sig_z = (y, -x, w, -z): +s on (x,z) from (y,w); -s on (y,w) from (x,z)
        nc.vector.scalar_tensor_tensor(pair(CD, 0, 2), pair(D, 1, 2), sh,
                                       pair(CD, 0, 2), op0=Op.mult, op1=Op.add)
        nc.vector.scalar_tensor_tensor(pair(CD, 1, 2), pair(D, 0, 2), -sh,
                                       pair(CD, 1, 2), op0=Op.mult, op1=Op.add)
    else:
        # sig_y = (-z, w, x, -y): x' -= s*z; y' += s*w; z' += s*x; w' -= s*y
        nc.vector.scalar_tensor_tensor(pair(CD, 1, 1), pair(D, 3, -3), sh,
                                       pair(CD, 1, 1), op0=Op.mult, op1=Op.add)
        nc.vector.scalar_tensor_tensor(pair(CD, 0, 3), pair(D, 2, -1), -sh,
                                       pair(CD, 0, 3), op0=Op.mult, op1=Op.add)
    return CD


def _var_rot(nc, st_pool, scrP_pool, dt, Op, D, T, axis):
    """D' = c*D + s*(D (x) axis_hat); T blocks (c, s, ns, s)."""
    Tap = T[:]
    tstride = Tap.ap[0][0]
    tbase = Tap.offset
    CD = st_pool.tile([P, 4, F], dt, tag="D")
    cb4 = T[:, 0:1, :].broadcast_to((P, 4, F))
    nc.vector.tensor_tensor(CD[:], D[:], cb4, op=Op.mult)
    Pt = scrP_pool.tile([P, 4, F], dt, tag="P")
    Dap = D[:]
    dstride = Dap.ap[0][0]
    out = _ap_with(Pt[:], Pt[:].offset,
                   [[Pt[:].ap[0][0], P], [2 * F, 2], [F, 2], [1, F]])
    if axis == 'z':
        fac = _ap_with(Tap, tbase + F, [[tstride, P], [0, 2], [F, 2], [1, F]])
        op = _ap_with(Dap, Dap.offset + F,
                      [[dstride, P], [2 * F, 2], [-F, 2], [1, F]])
    else:
        fac = _ap_with(Tap, tbase + 2 * F,
                       [[tstride, P], [F, 2], [-F, 2], [1, F]])
        op = _ap_with(Dap, Dap.offset + 2 * F,
                      [[dstride, P], [-2 * F, 2], [F, 2], [1, F]])
    nc.vector.tensor_tensor(out, fac, op, op=Op.mult)
    nc.vector.tensor_tensor(CD[:], CD[:], Pt[:], op=Op.add)
    return CD


# ------------------------------------------------------------------ public
def _get_program(rot_fixed, trans_fixed):
    key = (np.asarray(rot_fixed, np.float32).tobytes(),
           np.asarray(trans_fixed, np.float32).tobytes())
    if key not in _CACHE:
        links = _precompute(rot_fixed, trans_fixed)
        nc = _build_bass(links)
        kc = np.zeros((P, 36), np.float32)
        kc[:, 0] = PI / 2
        for i in range(1, N_LINKS):
            for c in range(3):
                kc[:, 1 + 3 * (i - 1) + c] = -links[i]['tf'][c]
        for i in range(N_LINKS):
            kc[:, 19 + i] = links[i]['hbias']
        kc[:, 26] = -1.0
        for c in range(3):
            kc[:, 27 + c] = links[0]['tf'][c]
        for c in range(3):
            kc[:, 30 + c] = links[1]['K'][c]
        _CACHE[key] = (nc, kc)
    return _CACHE[key]


def run(q, rot_fixed, trans_fixed, trace=False):
    from concourse.bass_utils import run_bass_kernel_spmd
    nc, kc = _get_program(rot_fixed, trans_fixed)
    q = np.asarray(q, np.float32)
    assert q.shape == (BATCH, N_LINKS), q.shape
    in_maps = []
    for c in range(N_CORES):
        qc = np.ascontiguousarray(q[c * PER_CORE:(c + 1) * PER_CORE].T)
        in_maps.append(dict(qt=qc, kcols=kc))
    res = run_bass_kernel_spmd(nc, in_maps, core_ids=list(range(N_CORES)),
                               trace=trace)
    ts = np.empty((N_LINKS, BATCH, 3), np.float32)
    qu = np.empty((N_LINKS, BATCH, 4), np.float32)
    for c in range(N_CORES):
        ts[:, c * PER_CORE:(c + 1) * PER_CORE] = res.results[c]["ts"]
        qu[:, c * PER_CORE:(c + 1) * PER_CORE] = res.results[c]["quats"]
    return (ts, qu), res


def kernel(q, rot_fixed, trans_fixed):
    (ts, qu), _ = run(q, rot_fixed, trans_fixed, trace=False)
    return ts, qu
